# revision 1
# baseline (speedup 1.0000x reference)
"""NetTGCN forward pass on 8 Trainium2 NeuronCores (Bass/Tile).

Sharding:
  Layer 1 (ChebTimeConv on the 4096-node graph): 4-way node-shard x 2-way
  batch-shard. The dense normalized adjacency (x2, transposed, bf16) stays
  resident in SBUF; each Chebyshev iteration all-gathers the new state
  across the 4 node shards (4-rank groups). Recurrence state is fp32.
  Transition: pooled features are redistributed with one 8-rank AllToAll
  so that layer 2 can run batch-parallel (core j owns batches
  {2j, 2j+1, 16+2j, 17+2j} - a mix of both batch halves, which makes every
  core's reads of the AllToAll output rank-uniform; the host unpermutes
  the final rows).
  Layer 2 (ChebConv on the 1024-node graph): batch-parallel, adjacency
  replicated. Head: h2 is transposed and all-gathered; fc1 is sharded over
  output columns (D) so reads are rank-uniform; z blocks are all-gathered
  and fc2 + log_softmax run redundantly on every core.

The FFT is folded into W1 on the host: real(FFT(x, axis=t)) = x @ Ccos and
Ccos commutes with the graph operator, so the recurrence runs on raw x
with W1_eff[k] = Ccos @ W1[k].
"""

import sys

if "/opt/trn_rl_repo" not in sys.path:
    sys.path.insert(0, "/opt/trn_rl_repo")

import numpy as np
import ml_dtypes

import concourse.bacc as bacc
import concourse.mybir as mybir
import concourse.bass_utils as _bu
from concourse.bass_utils import run_bass_kernel_spmd
from concourse.tile import TileContext
from concourse.masks import make_identity

_bu.upload_artifacts = lambda tmpdir: f"file://{tmpdir}"  # no bucket in sandbox

F32 = mybir.dt.float32
BF16 = mybir.dt.bfloat16
AX = mybir.AxisListType
ALU = mybir.AluOpType
ACT = mybir.ActivationFunctionType

B, N0, T, K = 32, 4096, 30, 25
G1, G2, D, C = 32, 64, 512, 10
N2 = N0 // 4
NCORES = 8
GCACHE = 12
NB = 4                 # layer-1 node shards
BL = B // 2            # 16 batches per layer-1 batch-half
TP = 32                # taps padded 30 -> 32
C1 = BL * TP           # 512 layer-1 channels per core
NBLK = N0 // NB        # 1024 nodes per layer-1 shard
P2BLK = N2 // NB       # 256 pooled nodes per layer-1 shard
B2 = 4                 # batches per layer-2 core
C2 = B2 * G1           # 128 layer-2 channels
DBLK = D // NCORES     # (unused) fc1 column split
FBLK = (N2 * G2) // NCORES  # 8192 fc1 contraction rows per core

G4 = [[0, 1, 2, 3], [4, 5, 6, 7]]
G8 = [list(range(NCORES))]


def _b16(a):
    return np.ascontiguousarray(a.astype(ml_dtypes.bfloat16))


def _dense_adj(edge_index, n):
    row = edge_index[0].astype(np.int64)
    col = edge_index[1].astype(np.int64)
    deg = np.zeros(n, np.float32)
    np.add.at(deg, row, 1.0)
    dis = np.where(deg > 0, 1.0 / np.sqrt(np.maximum(deg, 1.0)), 0.0).astype(np.float32)
    w = (-dis[row] * dis[col]).astype(np.float32)
    a = np.zeros((n, n), np.float32)
    np.add.at(a, (row, col), w)
    return a


def build_program(dbg=False):
    nc = bacc.Bacc("TRN2", target_bir_lowering=False, debug=False,
                   num_devices=NCORES)

    a1t_in = nc.dram_tensor("a1t", [N0, NBLK], BF16, kind="ExternalInput")
    m1t_in = nc.dram_tensor("m1t", [N0, NBLK], BF16, kind="ExternalInput")
    a2t_in = nc.dram_tensor("a2t", [N2, N2], BF16, kind="ExternalInput")
    x_nm_in = nc.dram_tensor("x_nm", [N0, C1], BF16, kind="ExternalInput")
    x_blk_in = nc.dram_tensor("x_blk", [NBLK, C1], F32, kind="ExternalInput")
    w1_in = nc.dram_tensor("w1a", [128, K * G1], BF16, kind="ExternalInput")
    w2_in = nc.dram_tensor("w2a", [128, K * 2 * G1], BF16, kind="ExternalInput")
    b1_in = nc.dram_tensor("b1v", [128, 1], F32, kind="ExternalInput")
    b2_in = nc.dram_tensor("b2v", [128, 2], F32, kind="ExternalInput")
    fc1w_in = nc.dram_tensor("fc1w", [FBLK, D], BF16, kind="ExternalInput")
    fc1b_in = nc.dram_tensor("fc1b", [B, D], F32, kind="ExternalInput")
    fc2w_in = nc.dram_tensor("fc2w", [D, C], BF16, kind="ExternalInput")
    fc2b_in = nc.dram_tensor("fc2b", [B, C], F32, kind="ExternalInput")

    out_t = nc.dram_tensor("out", [B, C], F32, kind="ExternalOutput")
    if dbg:
        h1_dbg = nc.dram_tensor("h1_dbg", [512, NBLK], F32, kind="ExternalOutput")
        l2i_dbg = nc.dram_tensor("l2i_dbg", [N2, C2], F32, kind="ExternalOutput")
        h2_dbg = nc.dram_tensor("h2_dbg", [256, N2], F32, kind="ExternalOutput")
        z_dbg = nc.dram_tensor("z_dbg", [B, D], F32, kind="ExternalOutput")

    cc1_in = [nc.dram_tensor(f"cc1i{i}", [NBLK, C1], BF16) for i in range(2)]
    cc1_out = [nc.dram_tensor(f"cc1o{i}", [N0, C1], BF16) for i in range(2)]
    ccp_in = nc.dram_tensor("ccp_in", [NCORES * P2BLK, 2 * G1], BF16)
    ccp_out = nc.dram_tensor("ccp_out", [NCORES * P2BLK, 2 * G1], BF16)
    cch_in = nc.dram_tensor("cch_in", [N2 * G2, B2], BF16)
    cch_out = nc.dram_tensor("cch_out", [N2 * G2, B2], BF16)
    ccz_in = nc.dram_tensor("ccz_in", [B, D], F32)
    ccz_out = nc.dram_tensor("ccz_out", [B, D], F32, addr_space="Shared")

    with TileContext(nc) as tc:
        with tc.tile_pool(name="const", bufs=1) as cpool:
            ident = cpool.tile([128, 128], F32)
            make_identity(nc, ident[:])

            # ======================= LAYER 1 =======================
            # Even/odd Chebyshev chains: T_{k} = 2*T_2*T_{k-2} - T_{k-4} with
            # M := 4*A^2 applied on PE and the -I part applied exactly on DVE:
            #   tx_k = M@tx_{k-2} - 2*tx_{k-2} - tx_{k-4}   (k >= 4)
            #   tx_2 = 0.5*M@tx_0 - tx_0 ;  tx_3 = M@tx_1 - 3*tx_1
            #   tx_1 = 0.5*(2A)@tx_0
            # Consecutive spmvs alternate chains, so the AllGather of chain X
            # overlaps the spmv of chain Y.
            # DRAM state rows are (p, t)-interleaved: stored row p*8+t holds
            # node t*128+p of the shard, so SBUF<->DRAM DMAs are contiguous.
            with tc.tile_pool(name="l1", bufs=1) as l1, \
                 tc.tile_pool(name="l1st", bufs=5) as l1st, \
                 tc.tile_pool(name="l1bf", bufs=1) as l1bf, \
                 tc.tile_pool(name="l1g", bufs=16) as l1g, \
                 tc.tile_pool(name="l1a", bufs=2) as l1a, \
                 tc.tile_pool(name="l1cm", bufs=1) as l1cm, \
                 tc.tile_pool(name="ps_y", bufs=1, space="PSUM") as ps_y, \
                 tc.tile_pool(name="ps_tr", bufs=2, space="PSUM") as ps_tr, \
                 tc.tile_pool(name="ps_ct", bufs=1, space="PSUM") as ps_ct:

                m1t = l1.tile([128, N0 // 128, NBLK], BF16)
                nc.sync.dma_start(m1t[:], m1t_in.ap().rearrange("(t p) n -> p t n", p=128))
                w1a = l1.tile([128, K, G1], BF16)
                nc.sync.dma_start(w1a[:], w1_in.ap().rearrange("p (k g) -> p k g", k=K))
                h1_sb = l1.tile([128, 4, NBLK], F32)
                nc.any.memset(h1_sb[:], 0.0)

                def l1_contract(src_f32, kk):
                    # src_f32: [128, 8, C1] fp32 node-major block state, term kk.
                    cm = l1cm.tile([128, 4, NBLK], BF16, tag="cm", name=f"cm{kk}")
                    for cht in range(4):
                        for ntg in range(2):
                            trt = ps_tr.tile([128, 4, 128], F32, tag="tr",
                                             name=f"tr{kk}_{cht}_{ntg}")
                            for j in range(4):
                                nt = 4 * ntg + j
                                nc.tensor.transpose(
                                    trt[:, j, :],
                                    src_f32[:, nt, 128 * cht:128 * (cht + 1)],
                                    ident[:])
                                nc.any.tensor_copy(
                                    out=cm[:, cht, 128 * nt:128 * (nt + 1)],
                                    in_=trt[:, j, :])
                    for cht in range(4):
                        for ch in range(NBLK // 512):
                            cps = ps_ct.tile([128, 512], F32, tag="ct",
                                             name=f"ct{kk}_{cht}_{ch}")
                            for bb in range(4):
                                nc.tensor.matmul(
                                    cps[32 * bb:32 * (bb + 1), :],
                                    w1a[32 * bb:32 * (bb + 1), kk, :],
                                    cm[32 * bb:32 * (bb + 1), cht, 512 * ch:512 * (ch + 1)],
                                    start=True, stop=True,
                                    tile_position=(32 * bb, 32 * bb))
                            nc.vector.tensor_tensor(
                                h1_sb[:, cht, 512 * ch:512 * (ch + 1)],
                                h1_sb[:, cht, 512 * ch:512 * (ch + 1)],
                                cps[:], ALU.add)

                NT0 = N0 // 128  # 32 gathered-node tiles

                def g_tile_ap(k, kt):
                    """[128, C1] stored-order tile kt of gathered term k."""
                    if k == 0:
                        base = x_nm_in.ap()
                    else:
                        base = cc1_out[k % 2].ap()
                    v = base.rearrange("(r p t) c -> r t p c", p=128, t=NBLK // 128)
                    return v[kt // (NBLK // 128), kt % (NBLK // 128)]

                tx_blk = {}
                tx_blk[0] = l1st.tile([128, NBLK // 128, C1], F32, tag="txs",
                                      name="txs0")
                nc.sync.dma_start(tx_blk[0][:],
                                  x_blk_in.ap().rearrange("(t p) c -> p t c", p=128))
                l1_contract(tx_blk[0], 0)

                for k in range(1, K):
                    gsrc = 0 if k <= 2 else k - 2
                    # stream gathered tiles; cache the last GCACHE for group 2
                    gk = {}
                    tx_new = l1st.tile([128, NBLK // 128, C1], F32, tag="txs",
                                       name=f"txs{k}")
                    txbf = l1bf.tile([128, NBLK // 128, C1], BF16, tag="txbf",
                                     name=f"txbf{k}")
                    for grp in range(2):
                        yp = ps_y.tile([128, 4, 512], F32, tag="y", name=f"y{k}_{grp}")
                        kts = (list(range(NT0)) if grp == 0
                               else list(range(NT0 - GCACHE, NT0))
                               + list(range(NT0 - GCACHE)))
                        for kt in kts:
                            if kt in gk:
                                gkt = gk.pop(kt)
                            else:
                                gkt = l1g.tile([128, C1], BF16, tag="gkt",
                                               name=f"g{k}_{grp}_{kt}")
                                nc.sync.dma_start(gkt[:], g_tile_ap(gsrc, kt))
                            if grp == 0 and kt >= NT0 - GCACHE:
                                gk[kt] = gkt
                            if k == 1:
                                op = l1a.tile([128, NBLK], BF16, tag="aop",
                                              name=f"a{grp}_{kt}")
                                nc.sync.dma_start(
                                    op[:], a1t_in.ap().rearrange(
                                        "(t p) n -> t p n", p=128)[kt])
                                opv = op[:, 512 * grp:512 * (grp + 1)]
                            else:
                                opv = m1t[:, kt, 512 * grp:512 * (grp + 1)]
                            for oi in range(4):
                                nc.tensor.matmul(
                                    yp[:, oi, :],
                                    opv[:, 128 * oi:128 * (oi + 1)],
                                    gkt[:],
                                    start=(kt == kts[0]), stop=(kt == kts[-1]))
                        for oi in range(4):
                            ot = 4 * grp + oi
                            yap = yp[:, oi, :]
                            o = tx_new[:, ot, :]
                            if k == 1:
                                nc.vector.tensor_scalar_mul(o, yap, 0.5)
                            elif k == 2:
                                nc.vector.tensor_scalar_mul(o, yap, 0.5)
                                nc.vector.tensor_tensor(o, o, tx_blk[0][:, ot, :],
                                                        ALU.subtract)
                            elif k == 3:
                                p1 = tx_blk[1][:, ot, :]
                                nc.vector.tensor_tensor(o, yap, p1, ALU.subtract)
                                nc.vector.tensor_tensor(o, o, p1, ALU.subtract)
                                nc.vector.tensor_tensor(o, o, p1, ALU.subtract)
                            else:
                                p2 = tx_blk[k - 2][:, ot, :]
                                nc.vector.tensor_tensor(o, yap, p2, ALU.subtract)
                                nc.vector.tensor_tensor(o, o, p2, ALU.subtract)
                                nc.vector.tensor_tensor(o, o, tx_blk[k - 4][:, ot, :],
                                                        ALU.subtract)
                            nc.vector.tensor_copy(txbf[:, ot, :], o)
                    tx_blk[k] = tx_new
                    # store in (p, t)-interleaved order, then 4-rank AllGather
                    cin, cout = cc1_in[k % 2], cc1_out[k % 2]
                    nc.sync.dma_start(
                        cin.ap().rearrange("(p t) c -> p t c", t=NBLK // 128),
                        txbf[:])
                    nc.gpsimd.collective_compute(
                        "AllGather", ALU.bypass, replica_groups=G4,
                        ins=[cin.ap()], outs=[cout.ap()])
                    l1_contract(tx_new, k)
                    tx_blk.pop(k - 4, None)

                # bias + relu + maxpool4 along nodes
                b1v = l1.tile([128, 1], F32)
                nc.sync.dma_start(b1v[:], b1_in.ap())
                h1p = l1.tile([128, 4, P2BLK], F32)
                for cht in range(4):
                    nc.scalar.activation(h1_sb[:, cht, :], h1_sb[:, cht, :], ACT.Relu,
                                         bias=b1v[:])
                    h4 = h1_sb[:, cht, :].rearrange("p (n f) -> p n f", f=4)
                    nc.vector.tensor_tensor(h1p[:, cht, :], h4[:, :, 0], h4[:, :, 1],
                                            ALU.max)
                    nc.vector.tensor_tensor(h1p[:, cht, :], h1p[:, cht, :], h4[:, :, 2],
                                            ALU.max)
                    nc.vector.tensor_tensor(h1p[:, cht, :], h1p[:, cht, :], h4[:, :, 3],
                                            ALU.max)
                if dbg:
                    nc.sync.dma_start(
                        h1_dbg.ap().rearrange("(t p) n -> p t n", p=128), h1_sb[:])

                # transpose pooled block -> [n2_local, (b_loc, g)] bf16
                h1pt = l1.tile([128, P2BLK // 128, BL * G1], BF16)
                for cht in range(4):
                    for nt in range(P2BLK // 128):
                        trp = ps_tr.tile([128, 128], F32, tag="tr")
                        nc.tensor.transpose(
                            trp[:], h1p[:, cht, 128 * nt:128 * (nt + 1)], ident[:])
                        nc.any.tensor_copy(
                            out=h1pt[:, nt, 128 * cht:128 * (cht + 1)], in_=trp[:])

                ccp_iv = ccp_in.ap().rearrange("(s t p) c -> s p t c", p=128,
                                               t=P2BLK // 128)
                for s in range(NCORES):
                    nc.sync.dma_start(ccp_iv[s],
                                      h1pt[:, :, 64 * s:64 * (s + 1)])
                nc.gpsimd.collective_compute(
                    "AllToAll", ALU.bypass, replica_groups=G8,
                    ins=[ccp_in.ap()], outs=[ccp_out.ap()])

            # ======================= LAYER 2 =======================
            # ccp_out rows: src_rank * P2BLK + n2l, src_rank = bh*4 + nb;
            # cols: (b_pair 2, g 32). My batches (c2 order): b = bh*2 + pair.
            with tc.tile_pool(name="l2", bufs=1) as l2, \
                 tc.tile_pool(name="l2st", bufs=3) as l2st, \
                 tc.tile_pool(name="l2bf", bufs=2) as l2bf, \
                 tc.tile_pool(name="l2cm", bufs=2) as l2cm, \
                 tc.tile_pool(name="ps2_y", bufs=2, space="PSUM") as ps2_y, \
                 tc.tile_pool(name="ps2_tr", bufs=2, space="PSUM") as ps2_tr, \
                 tc.tile_pool(name="ps2_ct", bufs=2, space="PSUM") as ps2_ct:

                a2t = l2.tile([128, N2 // 128, N2], BF16)
                nc.sync.dma_start(a2t[:], a2t_in.ap().rearrange("(t p) n -> p t n", p=128))
                w2a = l2.tile([128, K, 2, G1], BF16)
                nc.sync.dma_start(
                    w2a[:], w2_in.ap().rearrange("p (k h g) -> p k h g", k=K, h=2))

                # init state: [128 n2, 8 nt, (b 4, g 32)] from ccp_out
                st0_bf = l2bf.tile([128, N2 // 128, C2], BF16, tag="st2bf")
                ccp_v = ccp_out.ap().rearrange(
                    "(bh nb t p) c -> bh nb p t c", bh=2, nb=NB, t=P2BLK // 128)
                for bh in range(2):
                    for nb in range(NB):
                        # dest cols [bh*64, +64) = (b = bh*2 + pair, g)
                        nc.sync.dma_start(
                            st0_bf[:, 2 * nb:2 * (nb + 1),
                                   64 * bh:64 * (bh + 1)],
                            ccp_v[bh, nb])
                st0 = l2st.tile([128, N2 // 128, C2], F32, tag="st2")
                nc.vector.tensor_copy(st0[:], st0_bf[:])
                if dbg:
                    nc.sync.dma_start(
                        l2i_dbg.ap().rearrange("(t p) c -> p t c", p=128), st0[:])

                h2a = l2.tile([128, 2, N2], F32)
                nc.any.memset(h2a[:], 0.0)

                def l2_contract(src_f32, kk):
                    cm = l2cm.tile([128, N2], BF16, tag="cm2")
                    for nt in range(N2 // 128):
                        trp = ps2_tr.tile([128, 128], F32, tag="tr2")
                        nc.tensor.transpose(trp[:], src_f32[:, nt, :], ident[:])
                        nc.any.tensor_copy(
                            out=cm[:, 128 * nt:128 * (nt + 1)], in_=trp[:])
                    for hh in range(2):
                        cps = ps2_ct.tile([128, N2], F32, tag="ct2")
                        for ch in range(N2 // 512):
                            for bb in range(4):
                                nc.tensor.matmul(
                                    cps[32 * bb:32 * (bb + 1), 512 * ch:512 * (ch + 1)],
                                    w2a[32 * bb:32 * (bb + 1), kk, hh, :],
                                    cm[32 * bb:32 * (bb + 1), 512 * ch:512 * (ch + 1)],
                                    start=True, stop=True,
                                    tile_position=(32 * bb, 32 * bb))
                        nc.vector.tensor_tensor(h2a[:, hh, :], h2a[:, hh, :],
                                                cps[:], ALU.add)

                l2_contract(st0, 0)
                tx2_pp = None
                tx2_prev = st0
                gath2 = st0_bf
                for k in range(1, K):
                    yps = []
                    for g in range(2):
                        yp = ps2_y.tile([128, 4, 128], F32, tag="y2")
                        yps.append(yp)
                        for oi in range(4):
                            ot = 4 * g + oi
                            for kt in range(N2 // 128):
                                nc.tensor.matmul(
                                    yp[:, oi, :],
                                    a2t[:, kt, 128 * ot:128 * (ot + 1)],
                                    gath2[:, kt, :],
                                    start=(kt == 0), stop=(kt == N2 // 128 - 1))
                    tx2_new = l2st.tile([128, N2 // 128, C2], F32, tag="st2")
                    g2bf = l2bf.tile([128, N2 // 128, C2], BF16, tag="st2bf")
                    for ot in range(8):
                        yap = yps[ot // 4][:, ot % 4, :]
                        if k == 1:
                            nc.vector.tensor_scalar_mul(tx2_new[:, ot, :], yap, 0.5)
                        else:
                            nc.vector.tensor_tensor(tx2_new[:, ot, :], yap,
                                                    tx2_pp[:, ot, :], ALU.subtract)
                        nc.vector.tensor_copy(g2bf[:, ot, :], tx2_new[:, ot, :])
                    l2_contract(tx2_new, k)
                    gath2 = g2bf
                    tx2_pp = tx2_prev
                    tx2_prev = tx2_new

                # bias + relu, then transpose h2 -> [n2, (b, g2)] bf16
                b2v = l2.tile([128, 2], F32)
                nc.sync.dma_start(b2v[:], b2_in.ap())
                h2r = l2.tile([128, 2, N2], F32)
                for hh in range(2):
                    nc.scalar.activation(h2r[:, hh, :], h2a[:, hh, :], ACT.Relu,
                                         bias=b2v[:, hh:hh + 1])
                if dbg:
                    nc.sync.dma_start(
                        h2_dbg.ap().rearrange("(t p) n -> p t n", p=128), h2r[:])
                # build f-major features: ft_sb[n2_l, nt, (g2 64, b 4)]
                ft_sb = l2.tile([128, N2 // 128, G2 * B2], BF16)
                for hh in range(2):
                    for nt in range(N2 // 128):
                        trp = ps2_tr.tile([128, 128], F32, tag="tr2")
                        nc.tensor.transpose(trp[:], h2r[:, hh, 128 * nt:128 * (nt + 1)],
                                            ident[:])
                        # cols of trp: (b 4, g2r 32) -> dest (g2 = hh*32+g2r, b)
                        nc.any.tensor_copy(
                            out=ft_sb[:, nt, :].rearrange("p (g b) -> p g b", g=G2)[
                                :, 32 * hh:32 * (hh + 1), :],
                            in_=trp[:].rearrange("p (b g) -> p g b", b=4))
                # AllToAll: slot j = my rows f in [FBLK*j, FBLK*(j+1))
                # cch_in rows (j, n2_l 128, g2 64), cols b
                nc.sync.dma_start(
                    cch_in.ap().rearrange("(j nl g) b -> nl j (g b)",
                                          j=NCORES, nl=128),
                    ft_sb[:])
                nc.gpsimd.collective_compute(
                    "AllToAll", ALU.bypass, replica_groups=G8,
                    ins=[cch_in.ap()], outs=[cch_out.ap()])

            # ======================= HEAD =======================
            with tc.tile_pool(name="fc", bufs=1) as fc, \
                 tc.tile_pool(name="fcw", bufs=4) as fcw, \
                 tc.tile_pool(name="ps3", bufs=2, space="PSUM") as ps3, \
                 tc.tile_pool(name="ps3z", bufs=1, space="PSUM") as ps3z:

                # flatT: my f-block x all batches: [128 p, 64 kt, 32 (r 8, b 4)]
                flt = fc.tile([128, FBLK // 128, B], BF16, tag="flt")
                zps = ps3z.tile([32, D], F32)
                cch_v = cch_out.ap().rearrange(
                    "(r kt p) b -> r p kt b", r=NCORES, kt=FBLK // 128)
                for r in range(NCORES):
                    nc.sync.dma_start(flt[:, :, B2 * r:B2 * (r + 1)], cch_v[r])
                for kt in range(FBLK // 128):
                    fw = fcw.tile([128, D], BF16, tag="fw")
                    nc.sync.dma_start(
                        fw[:], fc1w_in.ap().rearrange("(kt p) d -> kt p d", p=128)[kt])
                    nc.tensor.matmul(zps[:], flt[:, kt, :], fw[:],
                                     start=(kt == 0), stop=(kt == FBLK // 128 - 1))
                zblk = fc.tile([32, D], F32)
                nc.vector.tensor_copy(zblk[:], zps[:])
                nc.sync.dma_start(ccz_in.ap(), zblk[:])
                nc.gpsimd.collective_compute(
                    "AllReduce", ALU.add, replica_groups=G8,
                    ins=[ccz_in.ap()], outs=[ccz_out.ap()])
                zfull = fc.tile([32, D], F32)
                nc.sync.dma_start(zfull[:], ccz_out.ap())
                zb = fc.tile([32, D], F32)
                nc.sync.dma_start(zb[:], fc1b_in.ap())
                nc.vector.tensor_tensor(zfull[:], zfull[:], zb[:], ALU.add)
                zr = fc.tile([32, D], F32)
                nc.scalar.activation(zr[:], zfull[:], ACT.Relu)
                if dbg:
                    nc.sync.dma_start(z_dbg.ap(), zr[:])

                # fc2: transpose z, then [32, 10] = sum_kt zT[kt].T @ fc2w[kt]
                f2w = fc.tile([128, 4, C], BF16)
                nc.sync.dma_start(f2w[:],
                                  fc2w_in.ap().rearrange("(t p) c -> p t c", p=128))
                lps = ps3.tile([32, C], F32, tag="lg")
                for t4 in range(4):
                    ztp = ps3.tile([128, 32], F32, tag="zt")
                    nc.tensor.transpose(ztp[:], zr[:, 128 * t4:128 * (t4 + 1)],
                                        ident[:32, :32])
                    zts = fc.tile([128, 32], BF16, tag="zts")
                    nc.any.tensor_copy(out=zts[:], in_=ztp[:])
                    nc.tensor.matmul(lps[:], zts[:], f2w[:, t4, :],
                                     start=(t4 == 0), stop=(t4 == 3))
                logits = fc.tile([32, C], F32)
                f2b = fc.tile([32, C], F32)
                nc.sync.dma_start(f2b[:], fc2b_in.ap())
                nc.vector.tensor_tensor(logits[:], lps[:], f2b[:], ALU.add)

                mx = fc.tile([32, 1], F32)
                nc.vector.tensor_reduce(mx[:], logits[:], axis=AX.X, op=ALU.max)
                sh = fc.tile([32, C], F32)
                nc.vector.tensor_tensor(sh[:], logits[:], mx[:].to_broadcast((32, C)),
                                        ALU.subtract)
                ex = fc.tile([32, C], F32)
                nc.scalar.activation(ex[:], sh[:], ACT.Exp)
                sm = fc.tile([32, 1], F32)
                nc.vector.tensor_reduce(sm[:], ex[:], axis=AX.X, op=ALU.add)
                lg = fc.tile([32, 1], F32)
                nc.scalar.activation(lg[:], sm[:], ACT.Ln)
                res = fc.tile([32, C], F32)
                nc.vector.tensor_tensor(res[:], sh[:], lg[:].to_broadcast((32, C)),
                                        ALU.subtract)
                nc.sync.dma_start(out_t.ap(), res[:])

    nc.compile()
    return nc


def _identity_cos():
    t = np.arange(T)
    f = np.arange(T)
    return np.cos(2.0 * np.pi * np.outer(t, f) / T).astype(np.float32)


def make_inputs(x, edge_index0, edge_index2, W1, b1, W2, b2,
                fc1_w, fc1_b, fc2_w, fc2_b):
    """Build the 8 per-core input maps."""
    A0 = _dense_adj(np.asarray(edge_index0), N0)
    A2 = _dense_adj(np.asarray(edge_index2), N2)
    A1T2 = _b16((2.0 * A0).T)              # [N0, N0] cols -> row blocks
    M1T = _b16((4.0 * (A0 @ A0)).T)        # 4*A^2, transposed
    A2T2 = _b16((2.0 * A2).T)              # [N2, N2]
    # (p, t)-interleaved storage order for gathered layer-1 state rows
    il = np.arange(N0)
    rank_, rem = il // NBLK, il % NBLK
    p_, t_ = rem // (NBLK // 128), rem % (NBLK // 128)
    node_of_row = rank_ * NBLK + t_ * 128 + p_

    Ccos = _identity_cos()
    W1e = np.einsum("tf,kfg->ktg", Ccos, np.asarray(W1, np.float32))  # [K, 30, G1]
    w1a = np.zeros((128, K, G1), np.float32)
    for bb in range(4):
        w1a[32 * bb:32 * bb + 30] = W1e.transpose(1, 0, 2)
    w1a = _b16(w1a.reshape(128, K * G1))

    W2f = np.asarray(W2, np.float32)       # [K, G1, G2]
    w2a = np.zeros((128, K, 2, G1), np.float32)
    for bb in range(4):
        for hh in range(2):
            w2a[32 * bb:32 * bb + 32, :, hh, :] = \
                W2f[:, :, 32 * hh:32 * hh + 32].transpose(1, 0, 2)
    w2a = _b16(w2a.reshape(128, K * 2 * G1))

    b1v = np.tile(np.asarray(b1, np.float32), 4).reshape(128, 1)
    b2f = np.asarray(b2, np.float32)
    b2v = np.stack([np.tile(b2f[:32], 4), np.tile(b2f[32:], 4)], 1).astype(np.float32)

    fc1b = np.tile(np.asarray(fc1_b, np.float32)[None, :], (B, 1))
    fc2b = np.tile(np.asarray(fc2_b, np.float32)[None, :], (B, 1))
    fc2w = _b16(np.asarray(fc2_w, np.float32))

    xf = np.asarray(x, np.float32)          # [B, N0, T]
    fc1wf = np.asarray(fc1_w, np.float32)   # [N2*G2, D]

    ins = []
    for core in range(NCORES):
        bh, nb = core // 4, core % 4
        # layer-1 channels: c = b_loc*32 + t, batches 16*bh + b_loc
        xs = xf[16 * bh:16 * (bh + 1)]          # [16, N0, 30]
        x_nm = np.zeros((N0, C1), np.float32)
        x_nm.reshape(N0, BL, TP)[:, :, :T] = xs.transpose(1, 0, 2)
        x_blk = x_nm[NBLK * nb:NBLK * (nb + 1)].copy()
        x_nm = x_nm[node_of_row]
        ins.append({
            "a1t": np.ascontiguousarray(A1T2[:, NBLK * nb:NBLK * (nb + 1)]),
            "m1t": np.ascontiguousarray(M1T[:, NBLK * nb:NBLK * (nb + 1)]),
            "a2t": A2T2,
            "x_nm": _b16(x_nm),
            "x_blk": x_blk,
            "w1a": w1a, "w2a": w2a, "b1v": b1v, "b2v": b2v,
            "fc1w": _b16(fc1wf[FBLK * core:FBLK * (core + 1), :]),
            "fc1b": fc1b, "fc2b": fc2b, "fc2w": fc2w,
        })
    return ins


def batch_perm():
    """flat row order (r, b_c2) -> global batch id."""
    perm = []
    for r in range(NCORES):
        for b_c2 in range(4):
            bh, pair = b_c2 // 2, b_c2 % 2
            perm.append(16 * bh + 2 * r + pair)
    return np.array(perm)


_CACHED = {}


def kernel(**inputs):
    if "nc" not in _CACHED:
        _CACHED["nc"] = build_program(dbg=False)
    nc = _CACHED["nc"]
    ins = make_inputs(**inputs)
    res = run_bass_kernel_spmd(nc, ins, core_ids=list(range(NCORES)))
    out = np.zeros((B, C), np.float32)
    out[batch_perm()] = res.results[0]["out"]
    return out



# revision 6
# speedup vs baseline: 1.6889x; 1.6889x over previous
"""NetTGCN forward pass on 8 Trainium2 NeuronCores (Bass/Tile).

Batch-parallel design, zero collectives until the fc head:
  Each core owns 4 batches. Layer-1 channels = 4 batches x 32 taps = 128 =
  exactly the SBUF partition width, so the full Chebyshev recurrence on the
  4096-node graph runs locally per core: state kept in SBUF in both
  [ch, node] (recurrence/contract) and node-major lhsT form (matmul
  stationary). The dense operator 2A^T (bf16, 33.5 MB) is split: 13 of 32
  contract row-tiles stay SBUF-resident, the other 19 are streamed from HBM
  per 512-column output slice (2.4 MB contiguous DMAs, hidden under the
  matmuls). Per Chebyshev term: 256 matmuls of [128x128]@[128x512] (~99% PE
  eff), 32 PE transposes to rebuild the lhsT form, and an inline W1[k]
  contraction into the fp32 h1 accumulator.
  The FFT is folded into W1 on the host (real(FFT(x)) = x @ Ccos commutes
  with the graph operator).
  Layer 2 (1024-node graph) is identical in structure with the 2 MB
  operator fully resident.
  Head: features are exchanged with one 8-rank AllToAll so each core
  contracts its 8192-row slice of fc1_w for all 32 batches; partial z is
  ReduceScattered (each core gets its own 4 batches), fc2 + log_softmax run
  locally, and the host concatenates per-core outputs.

States are bf16 throughout (validated on host: final rel err 6.4e-3 vs
6.1e-3 for fp32 states); accumulators (h1/h2/psum) are fp32.
"""

import sys

if "/opt/trn_rl_repo" not in sys.path:
    sys.path.insert(0, "/opt/trn_rl_repo")

import numpy as np
import ml_dtypes

import concourse.bacc as bacc
import concourse.mybir as mybir
import concourse.bass_utils as _bu
from concourse.bass_utils import run_bass_kernel_spmd
from concourse.tile import TileContext
from concourse.masks import make_identity

_bu.upload_artifacts = lambda tmpdir: f"file://{tmpdir}"  # no bucket in sandbox

F32 = mybir.dt.float32
BF16 = mybir.dt.bfloat16
AX = mybir.AxisListType
ALU = mybir.AluOpType
ACT = mybir.ActivationFunctionType

B, N0, T, K = 32, 4096, 30, 25
G1, G2, D, C = 32, 64, 512, 10
N2 = N0 // 4
NCORES = 8
BL = B // NCORES       # 4 batches per core
TP = 32                # taps padded 30 -> 32
CH = BL * TP           # 128 layer-1 channels = partition width
NT0 = N0 // 128        # 32 contract tiles (layer 1)
NRES = 13              # operator row-tiles resident in SBUF
NSTR = NT0 - NRES      # 19 streamed row-tiles
JRES = NRES * 128
SL = 512               # output slice width
NSL = N0 // SL         # 8 slices per term
NT2 = N2 // 128        # 8 contract tiles (layer 2)
FBLK = (N2 * G2) // NCORES  # 8192 fc1 contraction rows per core

G8 = [list(range(NCORES))]


def _b16(a):
    return np.ascontiguousarray(a.astype(ml_dtypes.bfloat16))


def _dense_adj(edge_index, n):
    row = edge_index[0].astype(np.int64)
    col = edge_index[1].astype(np.int64)
    deg = np.zeros(n, np.float32)
    np.add.at(deg, row, 1.0)
    dis = np.where(deg > 0, 1.0 / np.sqrt(np.maximum(deg, 1.0)), 0.0).astype(np.float32)
    w = (-dis[row] * dis[col]).astype(np.float32)
    a = np.zeros((n, n), np.float32)
    np.add.at(a, (row, col), w)
    return a


def build_program(dbg=False):
    nc = bacc.Bacc("TRN2", target_bir_lowering=False, debug=False,
                   num_devices=NCORES)

    at_res_in = nc.dram_tensor("at_res", [128 * NRES, N0], BF16, kind="ExternalInput")
    at_str_in = nc.dram_tensor("at_str", [NSL * 128 * NSTR, SL], BF16, kind="ExternalInput")
    x_cn_in = nc.dram_tensor("x_cn", [128, N0], BF16, kind="ExternalInput")
    x_lt_in = nc.dram_tensor("x_lt", [128 * NT0, CH], BF16, kind="ExternalInput")
    w1_in = nc.dram_tensor("w1a", [128, K * G1], BF16, kind="ExternalInput")
    b1_in = nc.dram_tensor("b1v", [128, 1], F32, kind="ExternalInput")
    a2t_in = nc.dram_tensor("a2t", [128 * NT2, N2], BF16, kind="ExternalInput")
    w2_in = nc.dram_tensor("w2a", [128, K * 2 * G1], BF16, kind="ExternalInput")
    b2_in = nc.dram_tensor("b2v", [128, 2], F32, kind="ExternalInput")
    fc1w_in = nc.dram_tensor("fc1w", [128 * (FBLK // 128), D], BF16, kind="ExternalInput")
    fc1b_in = nc.dram_tensor("fc1b", [BL, D], F32, kind="ExternalInput")
    fc2w_in = nc.dram_tensor("fc2w", [D, C], BF16, kind="ExternalInput")
    fc2b_in = nc.dram_tensor("fc2b", [BL, C], F32, kind="ExternalInput")

    out_t = nc.dram_tensor("out", [BL, C], F32, kind="ExternalOutput")
    if dbg:
        h1_dbg = nc.dram_tensor("h1_dbg", [128, N0], F32, kind="ExternalOutput")
        h1p_dbg = nc.dram_tensor("h1p_dbg", [128, N2], F32, kind="ExternalOutput")
        h2_dbg = nc.dram_tensor("h2_dbg", [128, 2 * N2], F32, kind="ExternalOutput")
        z_dbg = nc.dram_tensor("z_dbg", [BL, D], F32, kind="ExternalOutput")

    cch_in = nc.dram_tensor("cch_in", [NCORES * 128 * 64, BL], BF16)
    cch_out = nc.dram_tensor("cch_out", [NCORES * 128 * 64, BL], BF16)
    ccz_in = nc.dram_tensor("ccz_in", [B, D], F32)
    ccz_out = nc.dram_tensor("ccz_out", [BL, D], F32)

    with TileContext(nc) as tc:
        with tc.tile_pool(name="const", bufs=1) as cpool:
            identb = cpool.tile([128, 128], BF16)
            make_identity(nc, identb[:])
            identf = cpool.tile([128, 128], F32)
            make_identity(nc, identf[:])
            h1_sb = cpool.tile([128, N0], F32)
            h1p = cpool.tile([128, N2], F32)

            # ======================= LAYER 1 =======================
            with tc.tile_pool(name="l1a", bufs=1) as l1a, \
                 tc.tile_pool(name="l1s", bufs=3) as l1s, \
                 tc.tile_pool(name="l1st", bufs=1) as l1st, \
                 tc.tile_pool(name="ps_y", bufs=2, space="PSUM") as ps_y, \
                 tc.tile_pool(name="ps_tr", bufs=2, space="PSUM") as ps_tr, \
                 tc.tile_pool(name="ps_h", bufs=2, space="PSUM") as ps_h:

                at_res = l1a.tile([128, NRES, N0], BF16)
                at_res_v = at_res_in.ap().rearrange("(p t) n -> p t n", t=NRES)
                for t in range(NRES):
                    nc.sync.dma_start(at_res[:, t, :], at_res_v[:, t, :])
                w1a = l1a.tile([128, K, G1], BF16)
                nc.sync.dma_start(w1a[:], w1_in.ap().rearrange("p (k g) -> p k g", k=K))
                b1v = l1a.tile([128, 1], F32)
                nc.sync.dma_start(b1v[:], b1_in.ap())

                cn = [l1st.tile([128, N0], BF16, name=f"cn{i}", tag=f"cn{i}")
                      for i in range(3)]
                lt = [l1st.tile([128, NT0, CH], BF16, name=f"lt{i}", tag=f"lt{i}")
                      for i in range(2)]
                nc.sync.dma_start(cn[0][:], x_cn_in.ap())
                nc.sync.dma_start(lt[0][:],
                                  x_lt_in.ap().rearrange("(p t) c -> p t c", t=NT0))

                at_str_v = at_str_in.ap().rearrange(
                    "(g p t) n -> g p t n", g=NSL, p=128)

                def contract1(src_cn, kk, g, first):
                    sl = slice(SL * g, SL * (g + 1))
                    hp = ps_h.tile([128, SL], F32, tag="hp", name=f"hp{kk}_{g}")
                    for bb in range(4):
                        nc.tensor.matmul(
                            hp[32 * bb:32 * (bb + 1), :],
                            w1a[32 * bb:32 * (bb + 1), kk, :],
                            src_cn[32 * bb:32 * (bb + 1), sl],
                            start=True, stop=True,
                            tile_position=(32 * bb, 32 * bb))
                    if first:
                        nc.vector.tensor_copy(h1_sb[:, sl], hp[:])
                    else:
                        nc.vector.tensor_tensor(h1_sb[:, sl], h1_sb[:, sl],
                                                hp[:], ALU.add)

                for g in range(NSL):
                    contract1(cn[0], 0, g, True)

                for k in range(1, K):
                    cur_lt = lt[(k - 1) % 2]
                    new_lt = lt[k % 2]
                    new_cn = cn[k % 3]
                    prev_cn = cn[(k - 2) % 3] if k >= 2 else None
                    for g in range(NSL):
                        sl = slice(SL * g, SL * (g + 1))
                        sa = l1s.tile([128, 10, SL], BF16, tag="sa",
                                      name=f"sa{k}_{g}")
                        sb = l1s.tile([128, 10, SL], BF16, tag="sa",
                                      name=f"sb{k}_{g}")
                        nc.sync.dma_start(sa[:], at_str_v[g][:, 0:10, :])
                        nc.sync.dma_start(sb[:, 0:9, :], at_str_v[g][:, 10:19, :])
                        yp = ps_y.tile([128, SL], F32, tag="yp", name=f"yp{k}_{g}")
                        for jt in range(NRES):
                            nc.tensor.matmul(yp[:], cur_lt[:, jt, :],
                                             at_res[:, jt, sl],
                                             start=(jt == 0), stop=False)
                        for t in range(10):
                            nc.tensor.matmul(yp[:], cur_lt[:, NRES + t, :],
                                             sa[:, t, :],
                                             start=False, stop=False)
                        for t in range(9):
                            nc.tensor.matmul(yp[:], cur_lt[:, NRES + 10 + t, :],
                                             sb[:, t, :],
                                             start=False, stop=(t == 8))
                        if k == 1:
                            nc.vector.tensor_scalar_mul(new_cn[:, sl], yp[:], 0.5)
                        else:
                            nc.vector.tensor_tensor(new_cn[:, sl], yp[:],
                                                    prev_cn[:, sl], ALU.subtract)
                        for bq in range(4):
                            trp = ps_tr.tile([128, 128], BF16, tag="trp",
                                             name=f"trp{k}_{g}_{bq}")
                            nc.tensor.transpose(
                                trp[:],
                                new_cn[:, SL * g + 128 * bq:SL * g + 128 * (bq + 1)],
                                identb[:])
                            nc.vector.tensor_copy(new_lt[:, 4 * g + bq, :], trp[:])
                        contract1(new_cn, k, g, False)

                # bias + relu + maxpool4 along nodes
                for q in range(4):
                    nc.scalar.activation(h1_sb[:, 1024 * q:1024 * (q + 1)],
                                         h1_sb[:, 1024 * q:1024 * (q + 1)],
                                         ACT.Relu, bias=b1v[:])
                if dbg:
                    nc.sync.dma_start(h1_dbg.ap(), h1_sb[:])
                h4 = h1_sb[:].rearrange("p (n f) -> p n f", f=4)
                nc.vector.tensor_tensor(h1p[:], h4[:, :, 0], h4[:, :, 1], ALU.max)
                nc.vector.tensor_tensor(h1p[:], h1p[:], h4[:, :, 2], ALU.max)
                nc.vector.tensor_tensor(h1p[:], h1p[:], h4[:, :, 3], ALU.max)
                if dbg:
                    nc.sync.dma_start(h1p_dbg.ap(), h1p[:])

            # ======================= LAYER 2 =======================
            with tc.tile_pool(name="l2", bufs=1) as l2, \
                 tc.tile_pool(name="l2st", bufs=1) as l2st, \
                 tc.tile_pool(name="ps2y", bufs=2, space="PSUM") as ps2y, \
                 tc.tile_pool(name="ps2t", bufs=2, space="PSUM") as ps2t, \
                 tc.tile_pool(name="ps2h", bufs=2, space="PSUM") as ps2h, \
                 tc.tile_pool(name="psz", bufs=1, space="PSUM") as psz, \
                 tc.tile_pool(name="psl", bufs=1, space="PSUM") as psl:

                a2t = l2.tile([128, NT2, N2], BF16)
                a2t_v = a2t_in.ap().rearrange("(p t) n -> p t n", t=NT2)
                for t in range(NT2):
                    nc.sync.dma_start(a2t[:, t, :], a2t_v[:, t, :])
                w2a = l2.tile([128, K, 2, G1], BF16)
                nc.sync.dma_start(
                    w2a[:], w2_in.ap().rearrange("p (k h g) -> p k h g", k=K, h=2))
                b2v = l2.tile([128, 2], F32)
                nc.sync.dma_start(b2v[:], b2_in.ap())
                fcw = l2.tile([128, FBLK // 128, D], BF16)
                fcw_v = fc1w_in.ap().rearrange("(p t) d -> p t d", t=FBLK // 128)
                for q in range(8):
                    nc.sync.dma_start(fcw[:, 8 * q:8 * (q + 1), :],
                                      fcw_v[:, 8 * q:8 * (q + 1), :])

                cn2 = [l2st.tile([128, N2], BF16, name=f"cn2_{i}", tag=f"cn2_{i}")
                       for i in range(3)]
                lt2 = [l2st.tile([128, NT2, CH], BF16, name=f"lt2_{i}",
                                 tag=f"lt2_{i}") for i in range(2)]
                h2a = l2.tile([128, 2, N2], F32)

                nc.vector.tensor_copy(cn2[0][:], h1p[:])
                for nt in range(NT2):
                    trp = ps2t.tile([128, 128], BF16, tag="tr2", name=f"tr2i_{nt}")
                    nc.tensor.transpose(
                        trp[:], cn2[0][:, 128 * nt:128 * (nt + 1)], identb[:])
                    nc.vector.tensor_copy(lt2[0][:, nt, :], trp[:])

                def contract2(src_cn, kk, first):
                    for hh in range(2):
                        for g in range(2):
                            sl = slice(SL * g, SL * (g + 1))
                            hp = ps2h.tile([128, SL], F32, tag="hp2",
                                           name=f"hp2_{kk}_{hh}_{g}")
                            for bb in range(4):
                                nc.tensor.matmul(
                                    hp[32 * bb:32 * (bb + 1), :],
                                    w2a[32 * bb:32 * (bb + 1), kk, hh, :],
                                    src_cn[32 * bb:32 * (bb + 1), sl],
                                    start=True, stop=True,
                                    tile_position=(32 * bb, 32 * bb))
                            if first:
                                nc.vector.tensor_copy(h2a[:, hh, sl], hp[:])
                            else:
                                nc.vector.tensor_tensor(h2a[:, hh, sl],
                                                        h2a[:, hh, sl],
                                                        hp[:], ALU.add)

                contract2(cn2[0], 0, True)
                for k in range(1, K):
                    cur_lt2 = lt2[(k - 1) % 2]
                    new_lt2 = lt2[k % 2]
                    new_cn2 = cn2[k % 3]
                    prev_cn2 = cn2[(k - 2) % 3] if k >= 2 else None
                    for g in range(2):
                        sl = slice(SL * g, SL * (g + 1))
                        y2 = ps2y.tile([128, SL], F32, tag="y2",
                                       name=f"y2_{k}_{g}")
                        for jt in range(NT2):
                            nc.tensor.matmul(y2[:], cur_lt2[:, jt, :],
                                             a2t[:, jt, sl],
                                             start=(jt == 0), stop=(jt == NT2 - 1))
                        if k == 1:
                            nc.vector.tensor_scalar_mul(new_cn2[:, sl], y2[:], 0.5)
                        else:
                            nc.vector.tensor_tensor(new_cn2[:, sl], y2[:],
                                                    prev_cn2[:, sl], ALU.subtract)
                        for bq in range(4):
                            trp = ps2t.tile([128, 128], BF16, tag="tr2",
                                            name=f"tr2_{k}_{g}_{bq}")
                            nc.tensor.transpose(
                                trp[:],
                                new_cn2[:, SL * g + 128 * bq:SL * g + 128 * (bq + 1)],
                                identb[:])
                            nc.vector.tensor_copy(new_lt2[:, 4 * g + bq, :], trp[:])
                    contract2(new_cn2, k, False)

                # ======================= HEAD =======================
                h2r = l2.tile([128, 2, N2], F32)
                for hh in range(2):
                    nc.scalar.activation(h2r[:, hh, :], h2a[:, hh, :], ACT.Relu,
                                         bias=b2v[:, hh:hh + 1])
                if dbg:
                    nc.sync.dma_start(
                        h2_dbg.ap().rearrange("p (h n) -> p h n", h=2), h2r[:])

                # features to f-major: ft[n2l, nt, g2, b]
                ft = l2.tile([128, NT2, G2, BL], BF16)
                for hh in range(2):
                    for nt in range(NT2):
                        trp = ps2t.tile([128, 128], F32, tag="tr2",
                                        name=f"trh_{hh}_{nt}")
                        nc.tensor.transpose(
                            trp[:], h2r[:, hh, 128 * nt:128 * (nt + 1)], identf[:])
                        nc.any.tensor_copy(
                            out=ft[:, nt, 32 * hh:32 * (hh + 1), :],
                            in_=trp[:].rearrange("p (b g) -> p g b", b=BL))
                nc.sync.dma_start(
                    cch_in.ap().rearrange("(j nl g) b -> nl j (g b)",
                                          j=NCORES, nl=128),
                    ft[:])
                nc.gpsimd.collective_compute(
                    "AllToAll", ALU.bypass, replica_groups=G8,
                    ins=[cch_in.ap()], outs=[cch_out.ap()])

                # fc1 partial: z[32, D] for my f-slice
                flt = l2.tile([128, FBLK // 128, B], BF16)
                cch_v = cch_out.ap().rearrange(
                    "(r q p) b -> r p q b", r=NCORES, q=FBLK // 128)
                for r in range(NCORES):
                    nc.sync.dma_start(flt[:, :, BL * r:BL * (r + 1)], cch_v[r])
                zps = psz.tile([32, D], F32)
                for kt in range(FBLK // 128):
                    nc.tensor.matmul(zps[:], flt[:, kt, :], fcw[:, kt, :],
                                     start=(kt == 0), stop=(kt == FBLK // 128 - 1))
                zblk = l2.tile([32, D], F32)
                nc.vector.tensor_copy(zblk[:], zps[:])
                nc.sync.dma_start(ccz_in.ap(), zblk[:])
                nc.gpsimd.collective_compute(
                    "ReduceScatter", ALU.add, replica_groups=G8,
                    ins=[ccz_in.ap()], outs=[ccz_out.ap()])
                zfull = l2.tile([BL, D], F32)
                nc.sync.dma_start(zfull[:], ccz_out.ap())
                zb = l2.tile([BL, D], F32)
                nc.sync.dma_start(zb[:], fc1b_in.ap())
                nc.vector.tensor_tensor(zfull[:], zfull[:], zb[:], ALU.add)
                zr = l2.tile([BL, D], F32)
                nc.scalar.activation(zr[:], zfull[:], ACT.Relu)
                if dbg:
                    nc.sync.dma_start(z_dbg.ap(), zr[:])

                # fc2 + log_softmax on my 4 batches
                f2w = l2.tile([128, 4, C], BF16)
                nc.sync.dma_start(f2w[:],
                                  fc2w_in.ap().rearrange("(t p) c -> p t c", p=128))
                lps = psl.tile([BL, C], F32)
                for t4 in range(4):
                    ztp = ps2t.tile([128, BL], F32, tag="tr2", name=f"zt_{t4}")
                    nc.tensor.transpose(ztp[:], zr[:, 128 * t4:128 * (t4 + 1)],
                                        identf[:BL, :BL])
                    zts = l2.tile([128, BL], BF16, tag="zts", name=f"zts_{t4}")
                    nc.any.tensor_copy(out=zts[:], in_=ztp[:])
                    nc.tensor.matmul(lps[:], zts[:], f2w[:, t4, :],
                                     start=(t4 == 0), stop=(t4 == 3))
                logits = l2.tile([BL, C], F32)
                f2b = l2.tile([BL, C], F32)
                nc.sync.dma_start(f2b[:], fc2b_in.ap())
                nc.vector.tensor_tensor(logits[:], lps[:], f2b[:], ALU.add)

                mx = l2.tile([BL, 1], F32)
                nc.vector.tensor_reduce(mx[:], logits[:], axis=AX.X, op=ALU.max)
                sh = l2.tile([BL, C], F32)
                nc.vector.tensor_tensor(sh[:], logits[:],
                                        mx[:].to_broadcast((BL, C)), ALU.subtract)
                ex = l2.tile([BL, C], F32)
                nc.scalar.activation(ex[:], sh[:], ACT.Exp)
                sm = l2.tile([BL, 1], F32)
                nc.vector.tensor_reduce(sm[:], ex[:], axis=AX.X, op=ALU.add)
                lg = l2.tile([BL, 1], F32)
                nc.scalar.activation(lg[:], sm[:], ACT.Ln)
                res = l2.tile([BL, C], F32)
                nc.vector.tensor_tensor(res[:], sh[:],
                                        lg[:].to_broadcast((BL, C)), ALU.subtract)
                nc.sync.dma_start(out_t.ap(), res[:])

    nc.compile()
    return nc


def _identity_cos():
    t = np.arange(T)
    f = np.arange(T)
    return np.cos(2.0 * np.pi * np.outer(t, f) / T).astype(np.float32)


def make_inputs(x, edge_index0, edge_index2, W1, b1, W2, b2,
                fc1_w, fc1_b, fc2_w, fc2_b):
    """Build the 8 per-core input maps."""
    A0 = _dense_adj(np.asarray(edge_index0), N0)
    A2 = _dense_adj(np.asarray(edge_index2), N2)
    At1 = np.ascontiguousarray((2.0 * A0).T)   # [j, i] = 2*A0[i, j]
    At2 = np.ascontiguousarray((2.0 * A2).T)

    # resident: rows p*NRES+t = At1 row t*128+p
    at_res = _b16(At1[:JRES].reshape(NRES, 128, N0)
                  .transpose(1, 0, 2).reshape(128 * NRES, N0))
    s = At1[JRES:].reshape(NSTR, 128, N0).transpose(1, 0, 2)  # [p, t, n]
    at_str = np.stack([s[:, :, SL * g:SL * (g + 1)] for g in range(NSL)], 0)
    at_str = _b16(at_str.reshape(NSL * 128 * NSTR, SL))
    a2t = _b16(At2.reshape(NT2, 128, N2).transpose(1, 0, 2).reshape(128 * NT2, N2))

    Ccos = _identity_cos()
    W1e = np.einsum("tf,kfg->ktg", Ccos, np.asarray(W1, np.float32))  # [K, 30, G1]
    w1a = np.zeros((128, K, G1), np.float32)
    for bb in range(4):
        w1a[32 * bb:32 * bb + 30] = W1e.transpose(1, 0, 2)
    w1a = _b16(w1a.reshape(128, K * G1))

    W2f = np.asarray(W2, np.float32)       # [K, G1, G2]
    w2a = np.zeros((128, K, 2, G1), np.float32)
    for bb in range(4):
        for hh in range(2):
            w2a[32 * bb:32 * bb + 32, :, hh, :] = \
                W2f[:, :, 32 * hh:32 * hh + 32].transpose(1, 0, 2)
    w2a = _b16(w2a.reshape(128, K * 2 * G1))

    b1v = np.tile(np.asarray(b1, np.float32), 4).reshape(128, 1)
    b2f = np.asarray(b2, np.float32)
    b2v = np.stack([np.tile(b2f[:32], 4), np.tile(b2f[32:], 4)], 1).astype(np.float32)

    fc1b = np.tile(np.asarray(fc1_b, np.float32)[None, :], (BL, 1))
    fc2b = np.tile(np.asarray(fc2_b, np.float32)[None, :], (BL, 1))
    fc2w = _b16(np.asarray(fc2_w, np.float32))

    xf = np.asarray(x, np.float32)          # [B, N0, T]
    fc1wf = np.asarray(fc1_w, np.float32)   # [N2*G2, D]

    ins = []
    for core in range(NCORES):
        xs = xf[BL * core:BL * (core + 1)]          # [4, N0, 30]
        x_cn = np.zeros((BL, TP, N0), np.float32)
        x_cn[:, :T] = xs.transpose(0, 2, 1)
        x_cn = x_cn.reshape(CH, N0)                 # row = b*32 + t
        x_lt = x_cn.T.reshape(NT0, 128, CH).transpose(1, 0, 2).reshape(128 * NT0, CH)
        fc1w_r = fc1wf[FBLK * core:FBLK * (core + 1)]
        fc1w_r = fc1w_r.reshape(FBLK // 128, 128, D).transpose(1, 0, 2) \
                       .reshape(FBLK, D)
        ins.append({
            "at_res": at_res, "at_str": at_str, "a2t": a2t,
            "x_cn": _b16(x_cn), "x_lt": _b16(x_lt),
            "w1a": w1a, "w2a": w2a, "b1v": b1v, "b2v": b2v,
            "fc1w": _b16(fc1w_r), "fc1b": fc1b,
            "fc2w": fc2w, "fc2b": fc2b,
        })
    return ins


_CACHED = {}


def kernel(**inputs):
    if "nc" not in _CACHED:
        _CACHED["nc"] = build_program(dbg=False)
    nc = _CACHED["nc"]
    ins = make_inputs(**inputs)
    res = run_bass_kernel_spmd(nc, ins, core_ids=list(range(NCORES)))
    out = np.zeros((B, C), np.float32)
    for core in range(NCORES):
        out[BL * core:BL * (core + 1)] = res.results[core]["out"]
    return out


# revision 10
# speedup vs baseline: 1.7460x; 1.0338x over previous
"""NetTGCN forward pass on 8 Trainium2 NeuronCores (Bass/Tile).

Batch-parallel design, zero collectives until the fc head:
  Each core owns 4 batches. Layer-1 channels = 4 batches x 32 taps = 128 =
  exactly the SBUF partition width, so the full Chebyshev recurrence on the
  4096-node graph runs locally per core: state kept in SBUF in both
  [ch, node] (recurrence/contract) and node-major lhsT form (matmul
  stationary). The dense operator 2A^T (bf16, 33.5 MB) is split: 13 of 32
  contract row-tiles stay SBUF-resident, the other 19 are streamed from HBM
  per 512-column output slice (2.4 MB contiguous DMAs, hidden under the
  matmuls). Per Chebyshev term: 256 matmuls of [128x128]@[128x512] (~99% PE
  eff), 32 PE transposes to rebuild the lhsT form, and an inline W1[k]
  contraction into the fp32 h1 accumulator.
  The FFT is folded into W1 on the host (real(FFT(x)) = x @ Ccos commutes
  with the graph operator).
  Layer 2 (1024-node graph) is identical in structure with the 2 MB
  operator fully resident.
  Head: features are exchanged with one 8-rank AllToAll so each core
  contracts its 8192-row slice of fc1_w for all 32 batches; partial z is
  ReduceScattered (each core gets its own 4 batches), fc2 + log_softmax run
  locally, and the host concatenates per-core outputs.

States are bf16 throughout (validated on host: final rel err 6.4e-3 vs
6.1e-3 for fp32 states); accumulators (h1/h2/psum) are fp32.
"""

import sys

if "/opt/trn_rl_repo" not in sys.path:
    sys.path.insert(0, "/opt/trn_rl_repo")

import numpy as np
import ml_dtypes

import concourse.bacc as bacc
import concourse.mybir as mybir
import concourse.bass_utils as _bu
from concourse.bass_utils import run_bass_kernel_spmd
from concourse.tile import TileContext
from concourse.masks import make_identity

_bu.upload_artifacts = lambda tmpdir: f"file://{tmpdir}"  # no bucket in sandbox

F32 = mybir.dt.float32
BF16 = mybir.dt.bfloat16
AX = mybir.AxisListType
ALU = mybir.AluOpType
ACT = mybir.ActivationFunctionType

B, N0, T, K = 32, 4096, 30, 25
G1, G2, D, C = 32, 64, 512, 10
N2 = N0 // 4
NCORES = 8
BL = B // NCORES       # 4 batches per core
TP = 32                # taps padded 30 -> 32
CH = BL * TP           # 128 layer-1 channels = partition width
NT0 = N0 // 128        # 32 contract tiles (layer 1)
NRES = 13              # operator row-tiles resident in SBUF
NSTR = NT0 - NRES      # 19 streamed row-tiles
JRES = NRES * 128
SL = 512               # output slice width
NSL = N0 // SL         # 8 slices per term
NT2 = N2 // 128        # 8 contract tiles (layer 2)
FBLK = (N2 * G2) // NCORES  # 8192 fc1 contraction rows per core

G8 = [list(range(NCORES))]


def _b16(a):
    return np.ascontiguousarray(a.astype(ml_dtypes.bfloat16))


def _dense_adj(edge_index, n):
    row = edge_index[0].astype(np.int64)
    col = edge_index[1].astype(np.int64)
    deg = np.zeros(n, np.float32)
    np.add.at(deg, row, 1.0)
    dis = np.where(deg > 0, 1.0 / np.sqrt(np.maximum(deg, 1.0)), 0.0).astype(np.float32)
    w = (-dis[row] * dis[col]).astype(np.float32)
    a = np.zeros((n, n), np.float32)
    np.add.at(a, (row, col), w)
    return a


def build_program(dbg=False):
    nc = bacc.Bacc("TRN2", target_bir_lowering=False, debug=False,
                   num_devices=NCORES)

    at_res_in = nc.dram_tensor("at_res", [128 * NRES, N0], BF16, kind="ExternalInput")
    at_str_in = nc.dram_tensor("at_str", [NSL * 128 * NSTR, SL], BF16, kind="ExternalInput")
    x_cn_in = nc.dram_tensor("x_cn", [128, N0], BF16, kind="ExternalInput")
    x_lt_in = nc.dram_tensor("x_lt", [128 * NT0, CH], BF16, kind="ExternalInput")
    w1_in = nc.dram_tensor("w1a", [128, K * G1], BF16, kind="ExternalInput")
    b1_in = nc.dram_tensor("b1v", [128, 1], F32, kind="ExternalInput")
    a2t_in = nc.dram_tensor("a2t", [128 * NT2, N2], BF16, kind="ExternalInput")
    w2_in = nc.dram_tensor("w2a", [128, K * 2 * G1], BF16, kind="ExternalInput")
    b2_in = nc.dram_tensor("b2v", [128, 2], F32, kind="ExternalInput")
    fc1w_in = nc.dram_tensor("fc1w", [128 * (FBLK // 128), D], BF16, kind="ExternalInput")
    fc1b_in = nc.dram_tensor("fc1b", [BL, D], F32, kind="ExternalInput")
    fc2w_in = nc.dram_tensor("fc2w", [D, C], BF16, kind="ExternalInput")
    fc2b_in = nc.dram_tensor("fc2b", [BL, C], F32, kind="ExternalInput")

    out_t = nc.dram_tensor("out", [BL, C], F32, kind="ExternalOutput")
    if dbg:
        h1_dbg = nc.dram_tensor("h1_dbg", [128, N0], F32, kind="ExternalOutput")
        h1p_dbg = nc.dram_tensor("h1p_dbg", [128, N2], F32, kind="ExternalOutput")
        h2_dbg = nc.dram_tensor("h2_dbg", [128, 2 * N2], F32, kind="ExternalOutput")
        z_dbg = nc.dram_tensor("z_dbg", [BL, D], F32, kind="ExternalOutput")

    cch_in = nc.dram_tensor("cch_in", [NCORES * 128 * 64, BL], BF16)
    cch_out = nc.dram_tensor("cch_out", [NCORES * 128 * 64, BL], BF16)
    ccz_in = nc.dram_tensor("ccz_in", [B, D], F32)
    ccz_out = nc.dram_tensor("ccz_out", [BL, D], F32)

    with TileContext(nc) as tc:
        with tc.tile_pool(name="const", bufs=1) as cpool:
            identb = cpool.tile([128, 128], BF16)
            make_identity(nc, identb[:])
            identf = cpool.tile([128, 128], F32)
            make_identity(nc, identf[:])
            h1_sb = cpool.tile([128, N0], F32)
            h1p = cpool.tile([128, N2], F32)

            # ======================= LAYER 1 =======================
            with tc.tile_pool(name="l1a", bufs=1) as l1a, \
                 tc.tile_pool(name="l1s", bufs=3) as l1s, \
                 tc.tile_pool(name="l1st", bufs=1) as l1st, \
                 tc.tile_pool(name="ps_y", bufs=2, space="PSUM") as ps_y, \
                 tc.tile_pool(name="ps_tr", bufs=2, space="PSUM") as ps_tr, \
                 tc.tile_pool(name="ps_h", bufs=2, space="PSUM") as ps_h:

                at_res = l1a.tile([128, NRES, N0], BF16)
                at_res_v = at_res_in.ap().rearrange("(p t) n -> p t n", t=NRES)
                for t in range(NRES):
                    nc.sync.dma_start(at_res[:, t, :], at_res_v[:, t, :])
                w1a = l1a.tile([128, K, G1], BF16)
                nc.sync.dma_start(w1a[:], w1_in.ap().rearrange("p (k g) -> p k g", k=K))
                b1v = l1a.tile([128, 1], F32)
                nc.sync.dma_start(b1v[:], b1_in.ap())

                cn = [l1st.tile([128, N0], BF16, name=f"cn{i}", tag=f"cn{i}")
                      for i in range(3)]
                lt = [l1st.tile([128, NT0, CH], BF16, name=f"lt{i}", tag=f"lt{i}")
                      for i in range(2)]
                nc.sync.dma_start(cn[0][:], x_cn_in.ap())
                nc.sync.dma_start(lt[0][:],
                                  x_lt_in.ap().rearrange("(p t) c -> p t c", t=NT0))

                at_str_v = at_str_in.ap().rearrange(
                    "(g p t) n -> g p t n", g=NSL, p=128)

                def contract1(src_cn, kk, g, first):
                    sl = slice(SL * g, SL * (g + 1))
                    hp = ps_h.tile([128, SL], F32, tag="hp", name=f"hp{kk}_{g}")
                    for bb in range(4):
                        nc.tensor.matmul(
                            hp[32 * bb:32 * (bb + 1), :],
                            w1a[32 * bb:32 * (bb + 1), kk, :],
                            src_cn[32 * bb:32 * (bb + 1), sl],
                            start=True, stop=True,
                            tile_position=(32 * bb, 32 * bb))
                    if first:
                        nc.vector.tensor_copy(h1_sb[:, sl], hp[:])
                    else:
                        nc.vector.tensor_tensor(h1_sb[:, sl], h1_sb[:, sl],
                                                hp[:], ALU.add)

                for g in range(NSL):
                    contract1(cn[0], 0, g, True)

                def epi1(k, g):
                    """Transposes + h1 contract for slice (k, g); emitted
                    after the NEXT slice's matmuls so PE never stalls on
                    the DVE recurrence."""
                    new_cn = cn[k % 3]
                    new_lt = lt[k % 2]
                    for bq in range(4):
                        trp = ps_tr.tile([128, 128], BF16, tag="trp",
                                         name=f"trp{k}_{g}_{bq}")
                        nc.tensor.transpose(
                            trp[:],
                            new_cn[:, SL * g + 128 * bq:SL * g + 128 * (bq + 1)],
                            identb[:])
                        nc.vector.tensor_copy(new_lt[:, 4 * g + bq, :], trp[:])
                    contract1(new_cn, k, g, False)

                for k in range(1, K):
                    cur_lt = lt[(k - 1) % 2]
                    new_cn = cn[k % 3]
                    prev_cn = cn[(k - 2) % 3] if k >= 2 else None
                    for g in range(NSL):
                        sl = slice(SL * g, SL * (g + 1))
                        sa = l1s.tile([128, 10, SL], BF16, tag="sa",
                                      name=f"sa{k}_{g}")
                        sb = l1s.tile([128, 10, SL], BF16, tag="sa",
                                      name=f"sb{k}_{g}")
                        nc.sync.dma_start(sa[:], at_str_v[g][:, 0:10, :])
                        nc.sync.dma_start(sb[:, 0:9, :], at_str_v[g][:, 10:19, :])
                        yp = ps_y.tile([128, SL], F32, tag="yp", name=f"yp{k}_{g}")
                        for jt in range(NRES):
                            nc.tensor.matmul(yp[:], cur_lt[:, jt, :],
                                             at_res[:, jt, sl],
                                             start=(jt == 0), stop=False)
                        for t in range(10):
                            nc.tensor.matmul(yp[:], cur_lt[:, NRES + t, :],
                                             sa[:, t, :],
                                             start=False, stop=False)
                        for t in range(9):
                            nc.tensor.matmul(yp[:], cur_lt[:, NRES + 10 + t, :],
                                             sb[:, t, :],
                                             start=False, stop=(t == 8))
                        if k == 1:
                            nc.vector.tensor_scalar_mul(new_cn[:, sl], yp[:], 0.5)
                        else:
                            nc.vector.tensor_tensor(new_cn[:, sl], yp[:],
                                                    prev_cn[:, sl], ALU.subtract)
                        if g > 0:
                            epi1(k, g - 1)
                    epi1(k, NSL - 1)

                # bias + relu + maxpool4 along nodes
                for q in range(4):
                    nc.scalar.activation(h1_sb[:, 1024 * q:1024 * (q + 1)],
                                         h1_sb[:, 1024 * q:1024 * (q + 1)],
                                         ACT.Relu, bias=b1v[:])
                if dbg:
                    nc.sync.dma_start(h1_dbg.ap(), h1_sb[:])
                h4 = h1_sb[:].rearrange("p (n f) -> p n f", f=4)
                nc.vector.tensor_tensor(h1p[:], h4[:, :, 0], h4[:, :, 1], ALU.max)
                nc.vector.tensor_tensor(h1p[:], h1p[:], h4[:, :, 2], ALU.max)
                nc.vector.tensor_tensor(h1p[:], h1p[:], h4[:, :, 3], ALU.max)
                if dbg:
                    nc.sync.dma_start(h1p_dbg.ap(), h1p[:])

            # ======================= LAYER 2 =======================
            with tc.tile_pool(name="l2", bufs=1) as l2, \
                 tc.tile_pool(name="l2st", bufs=1) as l2st, \
                 tc.tile_pool(name="ps2y", bufs=2, space="PSUM") as ps2y, \
                 tc.tile_pool(name="ps2t", bufs=2, space="PSUM") as ps2t, \
                 tc.tile_pool(name="ps2h", bufs=2, space="PSUM") as ps2h, \
                 tc.tile_pool(name="psz", bufs=1, space="PSUM") as psz, \
                 tc.tile_pool(name="psl", bufs=1, space="PSUM") as psl:

                a2t = l2.tile([128, NT2, N2], BF16)
                a2t_v = a2t_in.ap().rearrange("(p t) n -> p t n", t=NT2)
                for t in range(NT2):
                    nc.sync.dma_start(a2t[:, t, :], a2t_v[:, t, :])
                w2a = l2.tile([128, K, 2, G1], BF16)
                nc.sync.dma_start(
                    w2a[:], w2_in.ap().rearrange("p (k h g) -> p k h g", k=K, h=2))
                b2v = l2.tile([128, 2], F32)
                nc.sync.dma_start(b2v[:], b2_in.ap())
                fcw = l2.tile([128, FBLK // 128, D], BF16)
                fcw_v = fc1w_in.ap().rearrange("(p t) d -> p t d", t=FBLK // 128)
                for q in range(8):
                    nc.sync.dma_start(fcw[:, 8 * q:8 * (q + 1), :],
                                      fcw_v[:, 8 * q:8 * (q + 1), :])

                cn2 = [l2st.tile([128, N2], BF16, name=f"cn2_{i}", tag=f"cn2_{i}")
                       for i in range(3)]
                lt2 = [l2st.tile([128, NT2, CH], BF16, name=f"lt2_{i}",
                                 tag=f"lt2_{i}") for i in range(2)]
                h2a = l2.tile([128, 2, N2], F32)

                nc.vector.tensor_copy(cn2[0][:], h1p[:])
                for nt in range(NT2):
                    trp = ps2t.tile([128, 128], BF16, tag="tr2", name=f"tr2i_{nt}")
                    nc.tensor.transpose(
                        trp[:], cn2[0][:, 128 * nt:128 * (nt + 1)], identb[:])
                    nc.vector.tensor_copy(lt2[0][:, nt, :], trp[:])

                def contract2(src_cn, kk, g, first):
                    sl = slice(SL * g, SL * (g + 1))
                    for hh in range(2):
                        hp = ps2h.tile([128, SL], F32, tag="hp2",
                                       name=f"hp2_{kk}_{hh}_{g}")
                        for bb in range(4):
                            nc.tensor.matmul(
                                hp[32 * bb:32 * (bb + 1), :],
                                w2a[32 * bb:32 * (bb + 1), kk, hh, :],
                                src_cn[32 * bb:32 * (bb + 1), sl],
                                start=True, stop=True,
                                tile_position=(32 * bb, 32 * bb))
                        if first:
                            nc.vector.tensor_copy(h2a[:, hh, sl], hp[:])
                        else:
                            nc.vector.tensor_tensor(h2a[:, hh, sl],
                                                    h2a[:, hh, sl],
                                                    hp[:], ALU.add)

                def epi2(k, g):
                    new_cn2 = cn2[k % 3]
                    new_lt2 = lt2[k % 2]
                    for bq in range(4):
                        trp = ps2t.tile([128, 128], BF16, tag="tr2",
                                        name=f"tr2_{k}_{g}_{bq}")
                        nc.tensor.transpose(
                            trp[:],
                            new_cn2[:, SL * g + 128 * bq:SL * g + 128 * (bq + 1)],
                            identb[:])
                        nc.vector.tensor_copy(new_lt2[:, 4 * g + bq, :], trp[:])
                    contract2(new_cn2, k, g, False)

                contract2(cn2[0], 0, 0, True)
                contract2(cn2[0], 0, 1, True)
                for k in range(1, K):
                    cur_lt2 = lt2[(k - 1) % 2]
                    new_cn2 = cn2[k % 3]
                    prev_cn2 = cn2[(k - 2) % 3] if k >= 2 else None
                    for g in range(2):
                        sl = slice(SL * g, SL * (g + 1))
                        y2 = ps2y.tile([128, SL], F32, tag="y2",
                                       name=f"y2_{k}_{g}")
                        for jt in range(NT2):
                            nc.tensor.matmul(y2[:], cur_lt2[:, jt, :],
                                             a2t[:, jt, sl],
                                             start=(jt == 0), stop=(jt == NT2 - 1))
                        if k == 1:
                            nc.vector.tensor_scalar_mul(new_cn2[:, sl], y2[:], 0.5)
                        else:
                            nc.vector.tensor_tensor(new_cn2[:, sl], y2[:],
                                                    prev_cn2[:, sl], ALU.subtract)
                        if g > 0:
                            epi2(k, g - 1)
                    epi2(k, 1)

                # ======================= HEAD =======================
                h2r = l2.tile([128, 2, N2], F32)
                for hh in range(2):
                    nc.scalar.activation(h2r[:, hh, :], h2a[:, hh, :], ACT.Relu,
                                         bias=b2v[:, hh:hh + 1])
                if dbg:
                    nc.sync.dma_start(
                        h2_dbg.ap().rearrange("p (h n) -> p h n", h=2), h2r[:])

                # features to f-major: ft[n2l, nt, g2, b]
                ft = l2.tile([128, NT2, G2, BL], BF16)
                for hh in range(2):
                    for nt in range(NT2):
                        trp = ps2t.tile([128, 128], F32, tag="tr2",
                                        name=f"trh_{hh}_{nt}")
                        nc.tensor.transpose(
                            trp[:], h2r[:, hh, 128 * nt:128 * (nt + 1)], identf[:])
                        nc.any.tensor_copy(
                            out=ft[:, nt, 32 * hh:32 * (hh + 1), :],
                            in_=trp[:].rearrange("p (b g) -> p g b", b=BL))
                nc.sync.dma_start(
                    cch_in.ap().rearrange("(j nl g) b -> nl j (g b)",
                                          j=NCORES, nl=128),
                    ft[:])
                nc.gpsimd.collective_compute(
                    "AllToAll", ALU.bypass, replica_groups=G8,
                    ins=[cch_in.ap()], outs=[cch_out.ap()])

                # fc1 partial: z[32, D] for my f-slice. Contract tiles are
                # indexed by g2 (64 of them), partition = n2l, so the
                # cch_out -> SBUF DMA is contiguous per partition.
                flt = l2.tile([128, G2, B], BF16)
                cch_v = cch_out.ap().rearrange(
                    "(r p q) b -> r p q b", r=NCORES, p=128)
                for r in range(NCORES):
                    nc.sync.dma_start(flt[:, :, BL * r:BL * (r + 1)], cch_v[r])
                zps = psz.tile([32, D], F32)
                for kt in range(G2):
                    nc.tensor.matmul(zps[:], flt[:, kt, :], fcw[:, kt, :],
                                     start=(kt == 0), stop=(kt == G2 - 1))
                zblk = l2.tile([32, D], F32)
                nc.vector.tensor_copy(zblk[:], zps[:])
                nc.sync.dma_start(ccz_in.ap(), zblk[:])
                nc.gpsimd.collective_compute(
                    "ReduceScatter", ALU.add, replica_groups=G8,
                    ins=[ccz_in.ap()], outs=[ccz_out.ap()])
                zfull = l2.tile([BL, D], F32)
                nc.sync.dma_start(zfull[:], ccz_out.ap())
                zb = l2.tile([BL, D], F32)
                nc.sync.dma_start(zb[:], fc1b_in.ap())
                nc.vector.tensor_tensor(zfull[:], zfull[:], zb[:], ALU.add)
                zr = l2.tile([BL, D], F32)
                nc.scalar.activation(zr[:], zfull[:], ACT.Relu)
                if dbg:
                    nc.sync.dma_start(z_dbg.ap(), zr[:])

                # fc2 + log_softmax on my 4 batches
                f2w = l2.tile([128, 4, C], BF16)
                nc.sync.dma_start(f2w[:],
                                  fc2w_in.ap().rearrange("(t p) c -> p t c", p=128))
                lps = psl.tile([BL, C], F32)
                for t4 in range(4):
                    ztp = ps2t.tile([128, BL], F32, tag="tr2", name=f"zt_{t4}")
                    nc.tensor.transpose(ztp[:], zr[:, 128 * t4:128 * (t4 + 1)],
                                        identf[:BL, :BL])
                    zts = l2.tile([128, BL], BF16, tag="zts", name=f"zts_{t4}")
                    nc.any.tensor_copy(out=zts[:], in_=ztp[:])
                    nc.tensor.matmul(lps[:], zts[:], f2w[:, t4, :],
                                     start=(t4 == 0), stop=(t4 == 3))
                logits = l2.tile([BL, C], F32)
                f2b = l2.tile([BL, C], F32)
                nc.sync.dma_start(f2b[:], fc2b_in.ap())
                nc.vector.tensor_tensor(logits[:], lps[:], f2b[:], ALU.add)

                mx = l2.tile([BL, 1], F32)
                nc.vector.tensor_reduce(mx[:], logits[:], axis=AX.X, op=ALU.max)
                sh = l2.tile([BL, C], F32)
                nc.vector.tensor_tensor(sh[:], logits[:],
                                        mx[:].to_broadcast((BL, C)), ALU.subtract)
                ex = l2.tile([BL, C], F32)
                nc.scalar.activation(ex[:], sh[:], ACT.Exp)
                sm = l2.tile([BL, 1], F32)
                nc.vector.tensor_reduce(sm[:], ex[:], axis=AX.X, op=ALU.add)
                lg = l2.tile([BL, 1], F32)
                nc.scalar.activation(lg[:], sm[:], ACT.Ln)
                res = l2.tile([BL, C], F32)
                nc.vector.tensor_tensor(res[:], sh[:],
                                        lg[:].to_broadcast((BL, C)), ALU.subtract)
                nc.sync.dma_start(out_t.ap(), res[:])

    nc.compile()
    return nc


def _identity_cos():
    t = np.arange(T)
    f = np.arange(T)
    return np.cos(2.0 * np.pi * np.outer(t, f) / T).astype(np.float32)


def make_inputs(x, edge_index0, edge_index2, W1, b1, W2, b2,
                fc1_w, fc1_b, fc2_w, fc2_b):
    """Build the 8 per-core input maps."""
    A0 = _dense_adj(np.asarray(edge_index0), N0)
    A2 = _dense_adj(np.asarray(edge_index2), N2)
    At1 = np.ascontiguousarray((2.0 * A0).T)   # [j, i] = 2*A0[i, j]
    At2 = np.ascontiguousarray((2.0 * A2).T)

    # resident: rows p*NRES+t = At1 row t*128+p
    at_res = _b16(At1[:JRES].reshape(NRES, 128, N0)
                  .transpose(1, 0, 2).reshape(128 * NRES, N0))
    s = At1[JRES:].reshape(NSTR, 128, N0).transpose(1, 0, 2)  # [p, t, n]
    at_str = np.stack([s[:, :, SL * g:SL * (g + 1)] for g in range(NSL)], 0)
    at_str = _b16(at_str.reshape(NSL * 128 * NSTR, SL))
    a2t = _b16(At2.reshape(NT2, 128, N2).transpose(1, 0, 2).reshape(128 * NT2, N2))

    Ccos = _identity_cos()
    W1e = np.einsum("tf,kfg->ktg", Ccos, np.asarray(W1, np.float32))  # [K, 30, G1]
    w1a = np.zeros((128, K, G1), np.float32)
    for bb in range(4):
        w1a[32 * bb:32 * bb + 30] = W1e.transpose(1, 0, 2)
    w1a = _b16(w1a.reshape(128, K * G1))

    W2f = np.asarray(W2, np.float32)       # [K, G1, G2]
    w2a = np.zeros((128, K, 2, G1), np.float32)
    for bb in range(4):
        for hh in range(2):
            w2a[32 * bb:32 * bb + 32, :, hh, :] = \
                W2f[:, :, 32 * hh:32 * hh + 32].transpose(1, 0, 2)
    w2a = _b16(w2a.reshape(128, K * 2 * G1))

    b1v = np.tile(np.asarray(b1, np.float32), 4).reshape(128, 1)
    b2f = np.asarray(b2, np.float32)
    b2v = np.stack([np.tile(b2f[:32], 4), np.tile(b2f[32:], 4)], 1).astype(np.float32)

    fc1b = np.tile(np.asarray(fc1_b, np.float32)[None, :], (BL, 1))
    fc2b = np.tile(np.asarray(fc2_b, np.float32)[None, :], (BL, 1))
    fc2w = _b16(np.asarray(fc2_w, np.float32))

    xf = np.asarray(x, np.float32)          # [B, N0, T]
    fc1wf = np.asarray(fc1_w, np.float32)   # [N2*G2, D]

    ins = []
    for core in range(NCORES):
        xs = xf[BL * core:BL * (core + 1)]          # [4, N0, 30]
        x_cn = np.zeros((BL, TP, N0), np.float32)
        x_cn[:, :T] = xs.transpose(0, 2, 1)
        x_cn = x_cn.reshape(CH, N0)                 # row = b*32 + t
        x_lt = x_cn.T.reshape(NT0, 128, CH).transpose(1, 0, 2).reshape(128 * NT0, CH)
        # fcw[p=n2l, kt=g2, d] = fc1_w[8192*core + n2l*64 + g2, d]: the
        # natural row-major order of the core's slice, no interleave.
        fc1w_r = fc1wf[FBLK * core:FBLK * (core + 1)]
        ins.append({
            "at_res": at_res, "at_str": at_str, "a2t": a2t,
            "x_cn": _b16(x_cn), "x_lt": _b16(x_lt),
            "w1a": w1a, "w2a": w2a, "b1v": b1v, "b2v": b2v,
            "fc1w": _b16(fc1w_r), "fc1b": fc1b,
            "fc2w": fc2w, "fc2b": fc2b,
        })
    return ins


_CACHED = {}


def kernel(**inputs):
    if "nc" not in _CACHED:
        _CACHED["nc"] = build_program(dbg=False)
    nc = _CACHED["nc"]
    ins = make_inputs(**inputs)
    res = run_bass_kernel_spmd(nc, ins, core_ids=list(range(NCORES)))
    out = np.zeros((B, C), np.float32)
    for core in range(NCORES):
        out[BL * core:BL * (core + 1)] = res.results[core]["out"]
    return out


# revision 17
# speedup vs baseline: 1.8106x; 1.0370x over previous
"""NetTGCN forward pass on 8 Trainium2 NeuronCores (Bass/Tile).

Batch-parallel design, zero collectives until the fc head:
  Each core owns 4 batches. Layer-1 channels = 4 batches x 32 taps = 128 =
  exactly the SBUF partition width, so the full Chebyshev recurrence on the
  4096-node graph runs locally per core: state kept in SBUF in both
  [ch, node] (recurrence/contract) and node-major lhsT form (matmul
  stationary). The dense operator 2A^T (bf16, 33.5 MB) is split: 13 of 32
  contract row-tiles stay SBUF-resident, the other 19 are streamed from HBM
  per 512-column output slice (2.4 MB contiguous DMAs, hidden under the
  matmuls). Per Chebyshev term: 256 matmuls of [128x128]@[128x512] (~99% PE
  eff), 32 PE transposes to rebuild the lhsT form, and an inline W1[k]
  contraction into the fp32 h1 accumulator.
  The FFT is folded into W1 on the host (real(FFT(x)) = x @ Ccos commutes
  with the graph operator).
  Layer 2 (1024-node graph) is identical in structure with the 2 MB
  operator fully resident.
  Head: features are exchanged with one 8-rank AllToAll so each core
  contracts its 8192-row slice of fc1_w for all 32 batches; partial z is
  ReduceScattered (each core gets its own 4 batches), fc2 + log_softmax run
  locally, and the host concatenates per-core outputs.

States are bf16 throughout (validated on host: final rel err 6.4e-3 vs
6.1e-3 for fp32 states); accumulators (h1/h2/psum) are fp32.
"""

import sys

if "/opt/trn_rl_repo" not in sys.path:
    sys.path.insert(0, "/opt/trn_rl_repo")

import numpy as np
import ml_dtypes

import concourse.bacc as bacc
import concourse.mybir as mybir
import concourse.bass_utils as _bu
from concourse.bass_utils import run_bass_kernel_spmd
from concourse.tile import TileContext
from concourse.masks import make_identity

_bu.upload_artifacts = lambda tmpdir: f"file://{tmpdir}"  # no bucket in sandbox

F32 = mybir.dt.float32
BF16 = mybir.dt.bfloat16
AX = mybir.AxisListType
ALU = mybir.AluOpType
ACT = mybir.ActivationFunctionType

B, N0, T, K = 32, 4096, 30, 25
G1, G2, D, C = 32, 64, 512, 10
N2 = N0 // 4
NCORES = 8
BL = B // NCORES       # 4 batches per core
TP = 32                # taps padded 30 -> 32
CH = BL * TP           # 128 layer-1 channels = partition width
NT0 = N0 // 128        # 32 contract tiles (layer 1)
NRES = 13              # operator row-tiles resident in SBUF
NSTR = NT0 - NRES      # 19 streamed row-tiles
JRES = NRES * 128
SL = 512               # output slice width
NSL = N0 // SL         # 8 slices per term
NT2 = N2 // 128        # 8 contract tiles (layer 2)
FBLK = (N2 * G2) // NCORES  # 8192 fc1 contraction rows per core

G8 = [list(range(NCORES))]


def _b16(a):
    return np.ascontiguousarray(a.astype(ml_dtypes.bfloat16))


def _dense_adj(edge_index, n):
    row = edge_index[0].astype(np.int64)
    col = edge_index[1].astype(np.int64)
    deg = np.zeros(n, np.float32)
    np.add.at(deg, row, 1.0)
    dis = np.where(deg > 0, 1.0 / np.sqrt(np.maximum(deg, 1.0)), 0.0).astype(np.float32)
    w = (-dis[row] * dis[col]).astype(np.float32)
    a = np.zeros((n, n), np.float32)
    np.add.at(a, (row, col), w)
    return a


def build_program(dbg=False):
    nc = bacc.Bacc("TRN2", target_bir_lowering=False, debug=False,
                   num_devices=NCORES)

    at_res_in = nc.dram_tensor("at_res", [128 * NRES, N0], BF16, kind="ExternalInput")
    at_str_in = nc.dram_tensor("at_str", [NSL * 128 * NSTR, SL], BF16, kind="ExternalInput")
    x_cn_in = nc.dram_tensor("x_cn", [128, N0], BF16, kind="ExternalInput")
    x_lt_in = nc.dram_tensor("x_lt", [128 * NT0, CH], BF16, kind="ExternalInput")
    w1_in = nc.dram_tensor("w1a", [128, K * G1], BF16, kind="ExternalInput")
    b1_in = nc.dram_tensor("b1v", [128, 1], F32, kind="ExternalInput")
    a2t_in = nc.dram_tensor("a2t", [128 * NT2, N2], BF16, kind="ExternalInput")
    w2_in = nc.dram_tensor("w2a", [128, K * 2 * G1], BF16, kind="ExternalInput")
    b2_in = nc.dram_tensor("b2v", [128, 2], F32, kind="ExternalInput")
    fc1w_in = nc.dram_tensor("fc1w", [128 * (FBLK // 128), D], BF16, kind="ExternalInput")
    fc1b_in = nc.dram_tensor("fc1b", [BL, D], F32, kind="ExternalInput")
    fc2w_in = nc.dram_tensor("fc2w", [D, C], BF16, kind="ExternalInput")
    fc2b_in = nc.dram_tensor("fc2b", [BL, C], F32, kind="ExternalInput")

    out_t = nc.dram_tensor("out", [BL, C], F32, kind="ExternalOutput")
    if dbg:
        h1_dbg = nc.dram_tensor("h1_dbg", [128, N0], F32, kind="ExternalOutput")
        h1p_dbg = nc.dram_tensor("h1p_dbg", [128, N2], F32, kind="ExternalOutput")
        h2_dbg = nc.dram_tensor("h2_dbg", [128, 2 * N2], F32, kind="ExternalOutput")
        z_dbg = nc.dram_tensor("z_dbg", [BL, D], F32, kind="ExternalOutput")

    cch_in = nc.dram_tensor("cch_in", [NCORES * 128 * 64, BL], BF16)
    cch_out = nc.dram_tensor("cch_out", [NCORES * 128 * 64, BL], BF16)
    ccz_in = nc.dram_tensor("ccz_in", [B, D], F32)
    ccz_out = nc.dram_tensor("ccz_out", [BL, D], F32)

    with TileContext(nc) as tc:
        with tc.tile_pool(name="const", bufs=1) as cpool:
            identb = cpool.tile([128, 128], BF16)
            make_identity(nc, identb[:])
            identf = cpool.tile([128, 128], F32)
            make_identity(nc, identf[:])
            h1_sb = cpool.tile([128, N0], F32)
            h1p = cpool.tile([128, N2], F32)

            # ======================= LAYER 1 =======================
            with tc.tile_pool(name="l1a", bufs=1) as l1a, \
                 tc.tile_pool(name="l1s", bufs=3) as l1s, \
                 tc.tile_pool(name="l1st", bufs=1) as l1st, \
                 tc.tile_pool(name="ps_y", bufs=2, space="PSUM") as ps_y, \
                 tc.tile_pool(name="ps_tr", bufs=4, space="PSUM") as ps_tr, \
                 tc.tile_pool(name="ps_h", bufs=2, space="PSUM") as ps_h:

                at_res = l1a.tile([128, NRES, N0], BF16)
                at_res_v = at_res_in.ap().rearrange("(p t) n -> p t n", t=NRES)
                for t in range(NRES):
                    nc.sync.dma_start(at_res[:, t, :], at_res_v[:, t, :])
                w1a = l1a.tile([128, K, G1], BF16)
                nc.sync.dma_start(w1a[:], w1_in.ap().rearrange("p (k g) -> p k g", k=K))
                b1v = l1a.tile([128, 1], F32)
                nc.sync.dma_start(b1v[:], b1_in.ap())

                cn = [l1st.tile([128, N0], BF16, name=f"cn{i}", tag=f"cn{i}")
                      for i in range(3)]
                lt = [l1st.tile([128, NT0, CH], BF16, name=f"lt{i}", tag=f"lt{i}")
                      for i in range(2)]
                nc.sync.dma_start(cn[0][:], x_cn_in.ap())
                nc.sync.dma_start(lt[0][:],
                                  x_lt_in.ap().rearrange("(p t) c -> p t c", t=NT0))

                at_str_v = at_str_in.ap().rearrange(
                    "(g p t) n -> g p t n", g=NSL, p=128)

                def contract1(src_cn, kk, g, first):
                    sl = slice(SL * g, SL * (g + 1))
                    hp = ps_h.tile([128, SL], F32, tag="hp", name=f"hp{kk}_{g}")
                    for bb in range(4):
                        nc.tensor.matmul(
                            hp[32 * bb:32 * (bb + 1), :],
                            w1a[32 * bb:32 * (bb + 1), kk, :],
                            src_cn[32 * bb:32 * (bb + 1), sl],
                            start=True, stop=True,
                            tile_position=(32 * bb, 32 * bb))
                    if first:
                        nc.vector.tensor_copy(h1_sb[:, sl], hp[:])
                    else:
                        nc.vector.tensor_tensor(h1_sb[:, sl], h1_sb[:, sl],
                                                hp[:], ALU.add)

                for g in range(NSL):
                    contract1(cn[0], 0, g, True)

                def epi1(k, g):
                    """Transposes + h1 contract for slice (k, g); emitted
                    after the NEXT slice's matmuls so PE never stalls on
                    the DVE recurrence."""
                    new_cn = cn[k % 3]
                    new_lt = lt[k % 2]
                    for bq in range(4):
                        trp = ps_tr.tile([128, 128], BF16, tag="trp",
                                         name=f"trp{k}_{g}_{bq}")
                        nc.tensor.transpose(
                            trp[:],
                            new_cn[:, SL * g + 128 * bq:SL * g + 128 * (bq + 1)],
                            identb[:])
                        nc.vector.tensor_copy(new_lt[:, 4 * g + bq, :], trp[:])
                    contract1(new_cn, k, g, False)

                for k in range(1, K):
                    cur_lt = lt[(k - 1) % 2]
                    new_cn = cn[k % 3]
                    prev_cn = cn[(k - 2) % 3] if k >= 2 else None
                    for g in range(NSL):
                        sl = slice(SL * g, SL * (g + 1))
                        sa = l1s.tile([128, 10, SL], BF16, tag="sa",
                                      name=f"sa{k}_{g}")
                        sb = l1s.tile([128, 10, SL], BF16, tag="sa",
                                      name=f"sb{k}_{g}")
                        nc.sync.dma_start(sa[:], at_str_v[g][:, 0:10, :])
                        nc.sync.dma_start(sb[:, 0:9, :], at_str_v[g][:, 10:19, :])
                        yp = ps_y.tile([128, SL], F32, tag="yp", name=f"yp{k}_{g}")
                        for jt in range(NRES):
                            nc.tensor.matmul(yp[:], cur_lt[:, jt, :],
                                             at_res[:, jt, sl],
                                             start=(jt == 0), stop=False)
                        for t in range(10):
                            nc.tensor.matmul(yp[:], cur_lt[:, NRES + t, :],
                                             sa[:, t, :],
                                             start=False, stop=False)
                        for t in range(9):
                            nc.tensor.matmul(yp[:], cur_lt[:, NRES + 10 + t, :],
                                             sb[:, t, :],
                                             start=False, stop=(t == 8))
                        if k == 1:
                            nc.vector.tensor_scalar_mul(new_cn[:, sl], yp[:], 0.5)
                        else:
                            nc.vector.tensor_tensor(new_cn[:, sl], yp[:],
                                                    prev_cn[:, sl], ALU.subtract)
                        if g > 0:
                            epi1(k, g - 1)
                    epi1(k, NSL - 1)

                # bias + relu + maxpool4 along nodes
                for q in range(4):
                    nc.scalar.activation(h1_sb[:, 1024 * q:1024 * (q + 1)],
                                         h1_sb[:, 1024 * q:1024 * (q + 1)],
                                         ACT.Relu, bias=b1v[:])
                if dbg:
                    nc.sync.dma_start(h1_dbg.ap(), h1_sb[:])
                h4 = h1_sb[:].rearrange("p (n f) -> p n f", f=4)
                nc.vector.tensor_tensor(h1p[:], h4[:, :, 0], h4[:, :, 1], ALU.max)
                nc.vector.tensor_tensor(h1p[:], h1p[:], h4[:, :, 2], ALU.max)
                nc.vector.tensor_tensor(h1p[:], h1p[:], h4[:, :, 3], ALU.max)
                if dbg:
                    nc.sync.dma_start(h1p_dbg.ap(), h1p[:])

            # ======================= LAYER 2 =======================
            with tc.tile_pool(name="l2", bufs=1) as l2, \
                 tc.tile_pool(name="l2st", bufs=1) as l2st, \
                 tc.tile_pool(name="ps2t", bufs=2, space="PSUM") as ps2t:

                a2t = l2.tile([128, NT2, N2], BF16)
                a2t_v = a2t_in.ap().rearrange("(p t) n -> p t n", t=NT2)
                for t in range(NT2):
                    nc.sync.dma_start(a2t[:, t, :], a2t_v[:, t, :])
                w2a = l2.tile([128, K, 2, G1], BF16)
                nc.sync.dma_start(
                    w2a[:], w2_in.ap().rearrange("p (k h g) -> p k h g", k=K, h=2))
                b2v = l2.tile([128, 2], F32)
                nc.sync.dma_start(b2v[:], b2_in.ap())
                fcw = l2.tile([128, FBLK // 128, D], BF16)
                fcw_v = fc1w_in.ap().rearrange("(p t) d -> p t d", t=FBLK // 128)
                for q in range(8):
                    nc.sync.dma_start(fcw[:, 8 * q:8 * (q + 1), :],
                                      fcw_v[:, 8 * q:8 * (q + 1), :])

                cn2 = [l2st.tile([128, N2], BF16, name=f"cn2_{i}", tag=f"cn2_{i}")
                       for i in range(3)]
                lt2 = [l2st.tile([128, NT2, CH], BF16, name=f"lt2_{i}",
                                 tag=f"lt2_{i}") for i in range(2)]
                h2r = l2.tile([128, 2, N2], F32)
                ft = l2.tile([128, NT2, G2, BL], BF16)

                with tc.tile_pool(name="ps2y", bufs=2, space="PSUM") as ps2y, \
                     tc.tile_pool(name="ps2h", bufs=1, space="PSUM") as ps2h:
                    # h2 accumulates in PSUM across all K terms: 4 banks,
                    # start at k=0, stop at k=K-1, relu reads PSUM directly.
                    h2ps = [[ps2h.tile([128, SL], F32, tag=f"h2ps_{hh}_{g}",
                                       name=f"h2ps_{hh}_{g}")
                             for g in range(2)] for hh in range(2)]

                    nc.vector.tensor_copy(cn2[0][:], h1p[:])
                    for nt in range(NT2):
                        trp = ps2t.tile([128, 128], BF16, tag="tr2",
                                        name=f"tr2i_{nt}")
                        nc.tensor.transpose(
                            trp[:], cn2[0][:, 128 * nt:128 * (nt + 1)], identb[:])
                        nc.vector.tensor_copy(lt2[0][:, nt, :], trp[:])

                    def contract2(src_cn, kk, g):
                        sl = slice(SL * g, SL * (g + 1))
                        for hh in range(2):
                            for bb in range(4):
                                nc.tensor.matmul(
                                    h2ps[hh][g][32 * bb:32 * (bb + 1), :],
                                    w2a[32 * bb:32 * (bb + 1), kk, hh, :],
                                    src_cn[32 * bb:32 * (bb + 1), sl],
                                    start=(kk == 0), stop=(kk == K - 1),
                                    tile_position=(32 * bb, 32 * bb))

                    def epi2(k, g):
                        new_cn2 = cn2[k % 3]
                        new_lt2 = lt2[k % 2]
                        for bq in range(4):
                            trp = ps2t.tile([128, 128], BF16, tag="tr2",
                                            name=f"tr2_{k}_{g}_{bq}")
                            nc.tensor.transpose(
                                trp[:],
                                new_cn2[:, SL * g + 128 * bq:SL * g + 128 * (bq + 1)],
                                identb[:])
                            nc.vector.tensor_copy(new_lt2[:, 4 * g + bq, :], trp[:])
                        contract2(new_cn2, k, g)

                    contract2(cn2[0], 0, 0)
                    contract2(cn2[0], 0, 1)
                    for k in range(1, K):
                        cur_lt2 = lt2[(k - 1) % 2]
                        new_cn2 = cn2[k % 3]
                        prev_cn2 = cn2[(k - 2) % 3] if k >= 2 else None
                        for g in range(2):
                            sl = slice(SL * g, SL * (g + 1))
                            y2 = ps2y.tile([128, SL], F32, tag="y2",
                                           name=f"y2_{k}_{g}")
                            for jt in range(NT2):
                                nc.tensor.matmul(y2[:], cur_lt2[:, jt, :],
                                                 a2t[:, jt, sl],
                                                 start=(jt == 0),
                                                 stop=(jt == NT2 - 1))
                            if k == 1:
                                nc.vector.tensor_scalar_mul(new_cn2[:, sl],
                                                            y2[:], 0.5)
                            else:
                                nc.vector.tensor_tensor(new_cn2[:, sl], y2[:],
                                                        prev_cn2[:, sl],
                                                        ALU.subtract)
                            if g > 0:
                                epi2(k, g - 1)
                        epi2(k, 1)

                    # ================== HEAD (part 1) ==================
                    for hh in range(2):
                        for g in range(2):
                            sl = slice(SL * g, SL * (g + 1))
                            nc.scalar.activation(h2r[:, hh, sl], h2ps[hh][g][:],
                                                 ACT.Relu, bias=b2v[:, hh:hh + 1])
                    if dbg:
                        nc.sync.dma_start(
                            h2_dbg.ap().rearrange("p (h n) -> p h n", h=2),
                            h2r[:])

                    # features to f-major: ft[n2l, nt, g2, b]
                    for hh in range(2):
                        for nt in range(NT2):
                            trp = ps2t.tile([128, 128], F32, tag="tr2",
                                            name=f"trh_{hh}_{nt}")
                            nc.tensor.transpose(
                                trp[:], h2r[:, hh, 128 * nt:128 * (nt + 1)],
                                identf[:])
                            nc.vector.tensor_copy(
                                out=ft[:, nt, 32 * hh:32 * (hh + 1), :],
                                in_=trp[:].rearrange("p (b g) -> p g b", b=BL))
                    nc.sync.dma_start(
                        cch_in.ap().rearrange("(j nl g) b -> nl j (g b)",
                                              j=NCORES, nl=128),
                        ft[:])
                    nc.gpsimd.collective_compute(
                        "AllToAll", ALU.bypass, replica_groups=G8,
                        ins=[cch_in.ap()], outs=[cch_out.ap()])

                # ================== HEAD (part 2) ==================
                # fc1 partial: z[32, D] for my f-slice. flt is
                # [p=n2l, r, q=g2, b] so each per-rank DMA from cch_out is
                # contiguous on both sides; the matmul stationary reads the
                # strided [p, (r, b)] AP per g2.
                with tc.tile_pool(name="psz", bufs=1, space="PSUM") as psz, \
                     tc.tile_pool(name="psl", bufs=1, space="PSUM") as psl:
                    head_tail(nc, tc, l2, ps2t, psz, psl, identf,
                              cch_out, ccz_in, ccz_out, fcw,
                              fc1b_in, fc2w_in, fc2b_in, out_t,
                              z_dbg if dbg else None)

    nc.compile()
    return nc


def head_tail(nc, tc, l2, ps2t, psz, psl, identf, cch_out, ccz_in, ccz_out,
              fcw, fc1b_in, fc2w_in, fc2b_in, out_t, z_dbg):
    """fc1 partial + ReduceScatter + fc2 + log_softmax."""
    if True:
        if True:
            if True:
                flt = l2.tile([128, NCORES, G2, BL], BF16)
                cch_v = cch_out.ap().rearrange(
                    "(r p q) b -> r p q b", r=NCORES, p=128)
                for r in range(NCORES):
                    nc.sync.dma_start(flt[:, r, :, :], cch_v[r])
                flt2 = l2.tile([128, G2, B], BF16)
                nc.vector.tensor_copy(
                    out=flt2[:].rearrange("p q (r b) -> p q r b", r=NCORES),
                    in_=flt[:].rearrange("p r q b -> p q r b"))
                zps = psz.tile([32, D], F32)
                for kt in range(G2):
                    nc.tensor.matmul(zps[:], flt2[:, kt, :], fcw[:, kt, :],
                                     start=(kt == 0), stop=(kt == G2 - 1))
                zblk = l2.tile([32, D], F32)
                nc.vector.tensor_copy(zblk[:], zps[:])
                nc.sync.dma_start(ccz_in.ap(), zblk[:])
                nc.gpsimd.collective_compute(
                    "ReduceScatter", ALU.add, replica_groups=G8,
                    ins=[ccz_in.ap()], outs=[ccz_out.ap()])
                zfull = l2.tile([BL, D], F32)
                nc.sync.dma_start(zfull[:], ccz_out.ap())
                zb = l2.tile([BL, D], F32)
                nc.sync.dma_start(zb[:], fc1b_in.ap())
                nc.vector.tensor_tensor(zfull[:], zfull[:], zb[:], ALU.add)
                zr = l2.tile([BL, D], F32)
                nc.scalar.activation(zr[:], zfull[:], ACT.Relu)
                if z_dbg is not None:
                    nc.sync.dma_start(z_dbg.ap(), zr[:])

                # fc2 + log_softmax on my 4 batches
                f2w = l2.tile([128, 4, C], BF16)
                nc.sync.dma_start(f2w[:],
                                  fc2w_in.ap().rearrange("(t p) c -> p t c", p=128))
                lps = psl.tile([BL, C], F32)
                for t4 in range(4):
                    ztp = ps2t.tile([128, BL], F32, tag="tr2", name=f"zt_{t4}")
                    nc.tensor.transpose(ztp[:], zr[:, 128 * t4:128 * (t4 + 1)],
                                        identf[:BL, :BL])
                    zts = l2.tile([128, BL], BF16, tag="zts", name=f"zts_{t4}")
                    nc.any.tensor_copy(out=zts[:], in_=ztp[:])
                    nc.tensor.matmul(lps[:], zts[:], f2w[:, t4, :],
                                     start=(t4 == 0), stop=(t4 == 3))
                logits = l2.tile([BL, C], F32)
                f2b = l2.tile([BL, C], F32)
                nc.sync.dma_start(f2b[:], fc2b_in.ap())
                nc.vector.tensor_tensor(logits[:], lps[:], f2b[:], ALU.add)

                mx = l2.tile([BL, 1], F32)
                nc.vector.tensor_reduce(mx[:], logits[:], axis=AX.X, op=ALU.max)
                sh = l2.tile([BL, C], F32)
                nc.vector.tensor_tensor(sh[:], logits[:],
                                        mx[:].to_broadcast((BL, C)), ALU.subtract)
                ex = l2.tile([BL, C], F32)
                nc.scalar.activation(ex[:], sh[:], ACT.Exp)
                sm = l2.tile([BL, 1], F32)
                nc.vector.tensor_reduce(sm[:], ex[:], axis=AX.X, op=ALU.add)
                lg = l2.tile([BL, 1], F32)
                nc.scalar.activation(lg[:], sm[:], ACT.Ln)
                res = l2.tile([BL, C], F32)
                nc.vector.tensor_tensor(res[:], sh[:],
                                        lg[:].to_broadcast((BL, C)), ALU.subtract)
                nc.sync.dma_start(out_t.ap(), res[:])


def _identity_cos():
    t = np.arange(T)
    f = np.arange(T)
    return np.cos(2.0 * np.pi * np.outer(t, f) / T).astype(np.float32)


def make_inputs(x, edge_index0, edge_index2, W1, b1, W2, b2,
                fc1_w, fc1_b, fc2_w, fc2_b):
    """Build the 8 per-core input maps."""
    A0 = _dense_adj(np.asarray(edge_index0), N0)
    A2 = _dense_adj(np.asarray(edge_index2), N2)
    At1 = np.ascontiguousarray((2.0 * A0).T)   # [j, i] = 2*A0[i, j]
    At2 = np.ascontiguousarray((2.0 * A2).T)

    # resident: rows p*NRES+t = At1 row t*128+p
    at_res = _b16(At1[:JRES].reshape(NRES, 128, N0)
                  .transpose(1, 0, 2).reshape(128 * NRES, N0))
    s = At1[JRES:].reshape(NSTR, 128, N0).transpose(1, 0, 2)  # [p, t, n]
    at_str = np.stack([s[:, :, SL * g:SL * (g + 1)] for g in range(NSL)], 0)
    at_str = _b16(at_str.reshape(NSL * 128 * NSTR, SL))
    a2t = _b16(At2.reshape(NT2, 128, N2).transpose(1, 0, 2).reshape(128 * NT2, N2))

    Ccos = _identity_cos()
    W1e = np.einsum("tf,kfg->ktg", Ccos, np.asarray(W1, np.float32))  # [K, 30, G1]
    w1a = np.zeros((128, K, G1), np.float32)
    for bb in range(4):
        w1a[32 * bb:32 * bb + 30] = W1e.transpose(1, 0, 2)
    w1a = _b16(w1a.reshape(128, K * G1))

    W2f = np.asarray(W2, np.float32)       # [K, G1, G2]
    w2a = np.zeros((128, K, 2, G1), np.float32)
    for bb in range(4):
        for hh in range(2):
            w2a[32 * bb:32 * bb + 32, :, hh, :] = \
                W2f[:, :, 32 * hh:32 * hh + 32].transpose(1, 0, 2)
    w2a = _b16(w2a.reshape(128, K * 2 * G1))

    b1v = np.tile(np.asarray(b1, np.float32), 4).reshape(128, 1)
    b2f = np.asarray(b2, np.float32)
    b2v = np.stack([np.tile(b2f[:32], 4), np.tile(b2f[32:], 4)], 1).astype(np.float32)

    fc1b = np.tile(np.asarray(fc1_b, np.float32)[None, :], (BL, 1))
    fc2b = np.tile(np.asarray(fc2_b, np.float32)[None, :], (BL, 1))
    fc2w = _b16(np.asarray(fc2_w, np.float32))

    xf = np.asarray(x, np.float32)          # [B, N0, T]
    fc1wf = np.asarray(fc1_w, np.float32)   # [N2*G2, D]

    ins = []
    for core in range(NCORES):
        xs = xf[BL * core:BL * (core + 1)]          # [4, N0, 30]
        x_cn = np.zeros((BL, TP, N0), np.float32)
        x_cn[:, :T] = xs.transpose(0, 2, 1)
        x_cn = x_cn.reshape(CH, N0)                 # row = b*32 + t
        x_lt = x_cn.T.reshape(NT0, 128, CH).transpose(1, 0, 2).reshape(128 * NT0, CH)
        # fcw[p=n2l, kt=g2, d] = fc1_w[8192*core + n2l*64 + g2, d]: the
        # natural row-major order of the core's slice, no interleave.
        fc1w_r = fc1wf[FBLK * core:FBLK * (core + 1)]
        ins.append({
            "at_res": at_res, "at_str": at_str, "a2t": a2t,
            "x_cn": _b16(x_cn), "x_lt": _b16(x_lt),
            "w1a": w1a, "w2a": w2a, "b1v": b1v, "b2v": b2v,
            "fc1w": _b16(fc1w_r), "fc1b": fc1b,
            "fc2w": fc2w, "fc2b": fc2b,
        })
    return ins


_CACHED = {}


def kernel(**inputs):
    if "nc" not in _CACHED:
        _CACHED["nc"] = build_program(dbg=False)
    nc = _CACHED["nc"]
    ins = make_inputs(**inputs)
    res = run_bass_kernel_spmd(nc, ins, core_ids=list(range(NCORES)))
    out = np.zeros((B, C), np.float32)
    for core in range(NCORES):
        out[BL * core:BL * (core + 1)] = res.results[core]["out"]
    return out


# revision 18
# speedup vs baseline: 1.8203x; 1.0053x over previous
"""NetTGCN forward pass on 8 Trainium2 NeuronCores (Bass/Tile).

Batch-parallel design, zero collectives until the fc head:
  Each core owns 4 batches. Layer-1 channels = 4 batches x 32 taps = 128 =
  exactly the SBUF partition width, so the full Chebyshev recurrence on the
  4096-node graph runs locally per core: state kept in SBUF in both
  [ch, node] (recurrence/contract) and node-major lhsT form (matmul
  stationary). The dense operator 2A^T (bf16, 33.5 MB) is split: 13 of 32
  contract row-tiles stay SBUF-resident, the other 19 are streamed from HBM
  per 512-column output slice (2.4 MB contiguous DMAs, hidden under the
  matmuls). Per Chebyshev term: 256 matmuls of [128x128]@[128x512] (~99% PE
  eff), 32 PE transposes to rebuild the lhsT form, and an inline W1[k]
  contraction into the fp32 h1 accumulator.
  The FFT is folded into W1 on the host (real(FFT(x)) = x @ Ccos commutes
  with the graph operator).
  Layer 2 (1024-node graph) is identical in structure with the 2 MB
  operator fully resident.
  Head: features are exchanged with one 8-rank AllToAll so each core
  contracts its 8192-row slice of fc1_w for all 32 batches; partial z is
  ReduceScattered (each core gets its own 4 batches), fc2 + log_softmax run
  locally, and the host concatenates per-core outputs.

States are bf16 throughout (validated on host: final rel err 6.4e-3 vs
6.1e-3 for fp32 states); accumulators (h1/h2/psum) are fp32.
"""

import sys

if "/opt/trn_rl_repo" not in sys.path:
    sys.path.insert(0, "/opt/trn_rl_repo")

import numpy as np
import ml_dtypes

import concourse.bacc as bacc
import concourse.mybir as mybir
import concourse.bass_utils as _bu
from concourse.bass_utils import run_bass_kernel_spmd
from concourse.tile import TileContext
from concourse.masks import make_identity

_bu.upload_artifacts = lambda tmpdir: f"file://{tmpdir}"  # no bucket in sandbox

F32 = mybir.dt.float32
BF16 = mybir.dt.bfloat16
AX = mybir.AxisListType
ALU = mybir.AluOpType
ACT = mybir.ActivationFunctionType

B, N0, T, K = 32, 4096, 30, 25
G1, G2, D, C = 32, 64, 512, 10
N2 = N0 // 4
NCORES = 8
BL = B // NCORES       # 4 batches per core
TP = 32                # taps padded 30 -> 32
CH = BL * TP           # 128 layer-1 channels = partition width
NT0 = N0 // 128        # 32 contract tiles (layer 1)
NRES = 13              # operator row-tiles resident in SBUF
NSTR = NT0 - NRES      # 19 streamed row-tiles
JRES = NRES * 128
SL = 512               # output slice width
NSL = N0 // SL         # 8 slices per term
NT2 = N2 // 128        # 8 contract tiles (layer 2)
FBLK = (N2 * G2) // NCORES  # 8192 fc1 contraction rows per core

G8 = [list(range(NCORES))]


def _b16(a):
    return np.ascontiguousarray(a.astype(ml_dtypes.bfloat16))


def _dense_adj(edge_index, n):
    row = edge_index[0].astype(np.int64)
    col = edge_index[1].astype(np.int64)
    deg = np.zeros(n, np.float32)
    np.add.at(deg, row, 1.0)
    dis = np.where(deg > 0, 1.0 / np.sqrt(np.maximum(deg, 1.0)), 0.0).astype(np.float32)
    w = (-dis[row] * dis[col]).astype(np.float32)
    a = np.zeros((n, n), np.float32)
    np.add.at(a, (row, col), w)
    return a


def build_program(dbg=False):
    nc = bacc.Bacc("TRN2", target_bir_lowering=False, debug=False,
                   num_devices=NCORES)

    at_res_in = nc.dram_tensor("at_res", [128 * NRES, N0], BF16, kind="ExternalInput")
    at_str_in = nc.dram_tensor("at_str", [NSL * 128 * NSTR, SL], BF16, kind="ExternalInput")
    x_cn_in = nc.dram_tensor("x_cn", [128, N0], BF16, kind="ExternalInput")
    x_lt_in = nc.dram_tensor("x_lt", [128 * NT0, CH], BF16, kind="ExternalInput")
    w1_in = nc.dram_tensor("w1a", [128, K * G1], BF16, kind="ExternalInput")
    b1_in = nc.dram_tensor("b1v", [128, 1], F32, kind="ExternalInput")
    a2t_in = nc.dram_tensor("a2t", [128 * NT2, N2], BF16, kind="ExternalInput")
    w2_in = nc.dram_tensor("w2a", [128, K * 2 * G1], BF16, kind="ExternalInput")
    b2_in = nc.dram_tensor("b2v", [128, 2], F32, kind="ExternalInput")
    fc1w_in = nc.dram_tensor("fc1w", [128 * (FBLK // 128), D], BF16, kind="ExternalInput")
    fc1b_in = nc.dram_tensor("fc1b", [BL, D], F32, kind="ExternalInput")
    fc2w_in = nc.dram_tensor("fc2w", [D, C], BF16, kind="ExternalInput")
    fc2b_in = nc.dram_tensor("fc2b", [BL, C], F32, kind="ExternalInput")

    out_t = nc.dram_tensor("out", [BL, C], F32, kind="ExternalOutput")
    if dbg:
        h1_dbg = nc.dram_tensor("h1_dbg", [128, N0], F32, kind="ExternalOutput")
        h1p_dbg = nc.dram_tensor("h1p_dbg", [128, N2], F32, kind="ExternalOutput")
        h2_dbg = nc.dram_tensor("h2_dbg", [128, 2 * N2], F32, kind="ExternalOutput")
        z_dbg = nc.dram_tensor("z_dbg", [BL, D], F32, kind="ExternalOutput")

    cch_in = nc.dram_tensor("cch_in", [NCORES * 128 * 64, BL], BF16)
    cch_out = nc.dram_tensor("cch_out", [NCORES * 128 * 64, BL], BF16)
    ccz_in = nc.dram_tensor("ccz_in", [B, D], F32)
    ccz_out = nc.dram_tensor("ccz_out", [BL, D], F32)

    with TileContext(nc) as tc:
        with tc.tile_pool(name="const", bufs=1) as cpool:
            identb = cpool.tile([128, 128], BF16)
            make_identity(nc, identb[:])
            identf = cpool.tile([128, 128], F32)
            make_identity(nc, identf[:])
            h1_sb = cpool.tile([128, N0], F32)
            h1p = cpool.tile([128, N2], F32)

            # ======================= LAYER 1 =======================
            with tc.tile_pool(name="l1a", bufs=1) as l1a, \
                 tc.tile_pool(name="l1s", bufs=3) as l1s, \
                 tc.tile_pool(name="l1st", bufs=1) as l1st, \
                 tc.tile_pool(name="ps_y", bufs=2, space="PSUM") as ps_y, \
                 tc.tile_pool(name="ps_tr", bufs=4, space="PSUM") as ps_tr, \
                 tc.tile_pool(name="ps_h", bufs=2, space="PSUM") as ps_h:

                # small inputs first: the DMA rings are FIFO, so the x /
                # weight loads must not queue behind 13 MB of operator tiles
                w1a = l1a.tile([128, K, G1], BF16)
                nc.sync.dma_start(w1a[:], w1_in.ap().rearrange("p (k g) -> p k g", k=K))
                b1v = l1a.tile([128, 1], F32)
                nc.sync.dma_start(b1v[:], b1_in.ap())
                cn = [l1st.tile([128, N0], BF16, name=f"cn{i}", tag=f"cn{i}")
                      for i in range(3)]
                lt = [l1st.tile([128, NT0, CH], BF16, name=f"lt{i}", tag=f"lt{i}")
                      for i in range(2)]
                nc.sync.dma_start(cn[0][:], x_cn_in.ap())
                nc.sync.dma_start(lt[0][:],
                                  x_lt_in.ap().rearrange("(p t) c -> p t c", t=NT0))

                at_res = l1a.tile([128, NRES, N0], BF16)
                at_res_v = at_res_in.ap().rearrange("(p t) n -> p t n", t=NRES)
                for t in range(NRES):
                    nc.sync.dma_start(at_res[:, t, :], at_res_v[:, t, :])

                at_str_v = at_str_in.ap().rearrange(
                    "(g p t) n -> g p t n", g=NSL, p=128)

                def contract1(src_cn, kk, g, first):
                    sl = slice(SL * g, SL * (g + 1))
                    hp = ps_h.tile([128, SL], F32, tag="hp", name=f"hp{kk}_{g}")
                    for bb in range(4):
                        nc.tensor.matmul(
                            hp[32 * bb:32 * (bb + 1), :],
                            w1a[32 * bb:32 * (bb + 1), kk, :],
                            src_cn[32 * bb:32 * (bb + 1), sl],
                            start=True, stop=True,
                            tile_position=(32 * bb, 32 * bb))
                    if first:
                        nc.vector.tensor_copy(h1_sb[:, sl], hp[:])
                    else:
                        nc.vector.tensor_tensor(h1_sb[:, sl], h1_sb[:, sl],
                                                hp[:], ALU.add)

                for g in range(NSL):
                    contract1(cn[0], 0, g, True)

                def epi1(k, g):
                    """Transposes + h1 contract for slice (k, g); emitted
                    after the NEXT slice's matmuls so PE never stalls on
                    the DVE recurrence."""
                    new_cn = cn[k % 3]
                    new_lt = lt[k % 2]
                    for bq in range(4):
                        trp = ps_tr.tile([128, 128], BF16, tag="trp",
                                         name=f"trp{k}_{g}_{bq}")
                        nc.tensor.transpose(
                            trp[:],
                            new_cn[:, SL * g + 128 * bq:SL * g + 128 * (bq + 1)],
                            identb[:])
                        nc.vector.tensor_copy(new_lt[:, 4 * g + bq, :], trp[:])
                    contract1(new_cn, k, g, False)

                for k in range(1, K):
                    cur_lt = lt[(k - 1) % 2]
                    new_cn = cn[k % 3]
                    prev_cn = cn[(k - 2) % 3] if k >= 2 else None
                    for g in range(NSL):
                        sl = slice(SL * g, SL * (g + 1))
                        sa = l1s.tile([128, 10, SL], BF16, tag="sa",
                                      name=f"sa{k}_{g}")
                        sb = l1s.tile([128, 10, SL], BF16, tag="sa",
                                      name=f"sb{k}_{g}")
                        nc.sync.dma_start(sa[:], at_str_v[g][:, 0:10, :])
                        nc.sync.dma_start(sb[:, 0:9, :], at_str_v[g][:, 10:19, :])
                        yp = ps_y.tile([128, SL], F32, tag="yp", name=f"yp{k}_{g}")
                        for jt in range(NRES):
                            nc.tensor.matmul(yp[:], cur_lt[:, jt, :],
                                             at_res[:, jt, sl],
                                             start=(jt == 0), stop=False)
                        for t in range(10):
                            nc.tensor.matmul(yp[:], cur_lt[:, NRES + t, :],
                                             sa[:, t, :],
                                             start=False, stop=False)
                        for t in range(9):
                            nc.tensor.matmul(yp[:], cur_lt[:, NRES + 10 + t, :],
                                             sb[:, t, :],
                                             start=False, stop=(t == 8))
                        if k == 1:
                            nc.vector.tensor_scalar_mul(new_cn[:, sl], yp[:], 0.5)
                        else:
                            nc.vector.tensor_tensor(new_cn[:, sl], yp[:],
                                                    prev_cn[:, sl], ALU.subtract)
                        if g > 0:
                            epi1(k, g - 1)
                    epi1(k, NSL - 1)

                # bias + relu + maxpool4 along nodes
                for q in range(4):
                    nc.scalar.activation(h1_sb[:, 1024 * q:1024 * (q + 1)],
                                         h1_sb[:, 1024 * q:1024 * (q + 1)],
                                         ACT.Relu, bias=b1v[:])
                if dbg:
                    nc.sync.dma_start(h1_dbg.ap(), h1_sb[:])
                h4 = h1_sb[:].rearrange("p (n f) -> p n f", f=4)
                nc.vector.tensor_tensor(h1p[:], h4[:, :, 0], h4[:, :, 1], ALU.max)
                nc.vector.tensor_tensor(h1p[:], h1p[:], h4[:, :, 2], ALU.max)
                nc.vector.tensor_tensor(h1p[:], h1p[:], h4[:, :, 3], ALU.max)
                if dbg:
                    nc.sync.dma_start(h1p_dbg.ap(), h1p[:])

            # ======================= LAYER 2 =======================
            with tc.tile_pool(name="l2", bufs=1) as l2, \
                 tc.tile_pool(name="l2st", bufs=1) as l2st, \
                 tc.tile_pool(name="ps2t", bufs=2, space="PSUM") as ps2t:

                a2t = l2.tile([128, NT2, N2], BF16)
                a2t_v = a2t_in.ap().rearrange("(p t) n -> p t n", t=NT2)
                for t in range(NT2):
                    nc.sync.dma_start(a2t[:, t, :], a2t_v[:, t, :])
                w2a = l2.tile([128, K, 2, G1], BF16)
                nc.sync.dma_start(
                    w2a[:], w2_in.ap().rearrange("p (k h g) -> p k h g", k=K, h=2))
                b2v = l2.tile([128, 2], F32)
                nc.sync.dma_start(b2v[:], b2_in.ap())
                fcw = l2.tile([128, FBLK // 128, D], BF16)
                fcw_v = fc1w_in.ap().rearrange("(p t) d -> p t d", t=FBLK // 128)
                for q in range(8):
                    nc.sync.dma_start(fcw[:, 8 * q:8 * (q + 1), :],
                                      fcw_v[:, 8 * q:8 * (q + 1), :])

                cn2 = [l2st.tile([128, N2], BF16, name=f"cn2_{i}", tag=f"cn2_{i}")
                       for i in range(3)]
                lt2 = [l2st.tile([128, NT2, CH], BF16, name=f"lt2_{i}",
                                 tag=f"lt2_{i}") for i in range(2)]
                h2r = l2.tile([128, 2, N2], F32)
                ft = l2.tile([128, NT2, G2, BL], BF16)

                with tc.tile_pool(name="ps2y", bufs=2, space="PSUM") as ps2y, \
                     tc.tile_pool(name="ps2h", bufs=1, space="PSUM") as ps2h:
                    # h2 accumulates in PSUM across all K terms: 4 banks,
                    # start at k=0, stop at k=K-1, relu reads PSUM directly.
                    h2ps = [[ps2h.tile([128, SL], F32, tag=f"h2ps_{hh}_{g}",
                                       name=f"h2ps_{hh}_{g}")
                             for g in range(2)] for hh in range(2)]

                    nc.vector.tensor_copy(cn2[0][:], h1p[:])
                    for nt in range(NT2):
                        trp = ps2t.tile([128, 128], BF16, tag="tr2",
                                        name=f"tr2i_{nt}")
                        nc.tensor.transpose(
                            trp[:], cn2[0][:, 128 * nt:128 * (nt + 1)], identb[:])
                        nc.vector.tensor_copy(lt2[0][:, nt, :], trp[:])

                    def contract2(src_cn, kk, g):
                        sl = slice(SL * g, SL * (g + 1))
                        for hh in range(2):
                            for bb in range(4):
                                nc.tensor.matmul(
                                    h2ps[hh][g][32 * bb:32 * (bb + 1), :],
                                    w2a[32 * bb:32 * (bb + 1), kk, hh, :],
                                    src_cn[32 * bb:32 * (bb + 1), sl],
                                    start=(kk == 0), stop=(kk == K - 1),
                                    tile_position=(32 * bb, 32 * bb))

                    def epi2(k, g):
                        new_cn2 = cn2[k % 3]
                        new_lt2 = lt2[k % 2]
                        for bq in range(4):
                            trp = ps2t.tile([128, 128], BF16, tag="tr2",
                                            name=f"tr2_{k}_{g}_{bq}")
                            nc.tensor.transpose(
                                trp[:],
                                new_cn2[:, SL * g + 128 * bq:SL * g + 128 * (bq + 1)],
                                identb[:])
                            nc.vector.tensor_copy(new_lt2[:, 4 * g + bq, :], trp[:])
                        contract2(new_cn2, k, g)

                    contract2(cn2[0], 0, 0)
                    contract2(cn2[0], 0, 1)
                    for k in range(1, K):
                        cur_lt2 = lt2[(k - 1) % 2]
                        new_cn2 = cn2[k % 3]
                        prev_cn2 = cn2[(k - 2) % 3] if k >= 2 else None
                        for g in range(2):
                            sl = slice(SL * g, SL * (g + 1))
                            y2 = ps2y.tile([128, SL], F32, tag="y2",
                                           name=f"y2_{k}_{g}")
                            for jt in range(NT2):
                                nc.tensor.matmul(y2[:], cur_lt2[:, jt, :],
                                                 a2t[:, jt, sl],
                                                 start=(jt == 0),
                                                 stop=(jt == NT2 - 1))
                            if k == 1:
                                nc.vector.tensor_scalar_mul(new_cn2[:, sl],
                                                            y2[:], 0.5)
                            else:
                                nc.vector.tensor_tensor(new_cn2[:, sl], y2[:],
                                                        prev_cn2[:, sl],
                                                        ALU.subtract)
                            if g > 0:
                                epi2(k, g - 1)
                        epi2(k, 1)

                    # ================== HEAD (part 1) ==================
                    for hh in range(2):
                        for g in range(2):
                            sl = slice(SL * g, SL * (g + 1))
                            nc.scalar.activation(h2r[:, hh, sl], h2ps[hh][g][:],
                                                 ACT.Relu, bias=b2v[:, hh:hh + 1])
                    if dbg:
                        nc.sync.dma_start(
                            h2_dbg.ap().rearrange("p (h n) -> p h n", h=2),
                            h2r[:])

                    # features to f-major: ft[n2l, nt, g2, b]
                    for hh in range(2):
                        for nt in range(NT2):
                            trp = ps2t.tile([128, 128], F32, tag="tr2",
                                            name=f"trh_{hh}_{nt}")
                            nc.tensor.transpose(
                                trp[:], h2r[:, hh, 128 * nt:128 * (nt + 1)],
                                identf[:])
                            nc.vector.tensor_copy(
                                out=ft[:, nt, 32 * hh:32 * (hh + 1), :],
                                in_=trp[:].rearrange("p (b g) -> p g b", b=BL))
                    nc.sync.dma_start(
                        cch_in.ap().rearrange("(j nl g) b -> nl j (g b)",
                                              j=NCORES, nl=128),
                        ft[:])
                    nc.gpsimd.collective_compute(
                        "AllToAll", ALU.bypass, replica_groups=G8,
                        ins=[cch_in.ap()], outs=[cch_out.ap()])

                # ================== HEAD (part 2) ==================
                # fc1 partial: z[32, D] for my f-slice. flt is
                # [p=n2l, r, q=g2, b] so each per-rank DMA from cch_out is
                # contiguous on both sides; the matmul stationary reads the
                # strided [p, (r, b)] AP per g2.
                with tc.tile_pool(name="psz", bufs=1, space="PSUM") as psz, \
                     tc.tile_pool(name="psl", bufs=1, space="PSUM") as psl:
                    head_tail(nc, tc, l2, ps2t, psz, psl, identf,
                              cch_out, ccz_in, ccz_out, fcw,
                              fc1b_in, fc2w_in, fc2b_in, out_t,
                              z_dbg if dbg else None)

    nc.compile()
    return nc


def head_tail(nc, tc, l2, ps2t, psz, psl, identf, cch_out, ccz_in, ccz_out,
              fcw, fc1b_in, fc2w_in, fc2b_in, out_t, z_dbg):
    """fc1 partial + ReduceScatter + fc2 + log_softmax."""
    if True:
        if True:
            if True:
                flt = l2.tile([128, NCORES, G2, BL], BF16)
                cch_v = cch_out.ap().rearrange(
                    "(r p q) b -> r p q b", r=NCORES, p=128)
                for r in range(NCORES):
                    nc.sync.dma_start(flt[:, r, :, :], cch_v[r])
                flt2 = l2.tile([128, G2, B], BF16)
                nc.vector.tensor_copy(
                    out=flt2[:].rearrange("p q (r b) -> p q r b", r=NCORES),
                    in_=flt[:].rearrange("p r q b -> p q r b"))
                zps = psz.tile([32, D], F32)
                for kt in range(G2):
                    nc.tensor.matmul(zps[:], flt2[:, kt, :], fcw[:, kt, :],
                                     start=(kt == 0), stop=(kt == G2 - 1))
                zblk = l2.tile([32, D], F32)
                nc.vector.tensor_copy(zblk[:], zps[:])
                nc.sync.dma_start(ccz_in.ap(), zblk[:])
                nc.gpsimd.collective_compute(
                    "ReduceScatter", ALU.add, replica_groups=G8,
                    ins=[ccz_in.ap()], outs=[ccz_out.ap()])
                zfull = l2.tile([BL, D], F32)
                nc.sync.dma_start(zfull[:], ccz_out.ap())
                zb = l2.tile([BL, D], F32)
                nc.sync.dma_start(zb[:], fc1b_in.ap())
                nc.vector.tensor_tensor(zfull[:], zfull[:], zb[:], ALU.add)
                zr = l2.tile([BL, D], F32)
                nc.scalar.activation(zr[:], zfull[:], ACT.Relu)
                if z_dbg is not None:
                    nc.sync.dma_start(z_dbg.ap(), zr[:])

                # fc2 + log_softmax on my 4 batches
                f2w = l2.tile([128, 4, C], BF16)
                nc.sync.dma_start(f2w[:],
                                  fc2w_in.ap().rearrange("(t p) c -> p t c", p=128))
                lps = psl.tile([BL, C], F32)
                for t4 in range(4):
                    ztp = ps2t.tile([128, BL], F32, tag="tr2", name=f"zt_{t4}")
                    nc.tensor.transpose(ztp[:], zr[:, 128 * t4:128 * (t4 + 1)],
                                        identf[:BL, :BL])
                    zts = l2.tile([128, BL], BF16, tag="zts", name=f"zts_{t4}")
                    nc.any.tensor_copy(out=zts[:], in_=ztp[:])
                    nc.tensor.matmul(lps[:], zts[:], f2w[:, t4, :],
                                     start=(t4 == 0), stop=(t4 == 3))
                logits = l2.tile([BL, C], F32)
                f2b = l2.tile([BL, C], F32)
                nc.sync.dma_start(f2b[:], fc2b_in.ap())
                nc.vector.tensor_tensor(logits[:], lps[:], f2b[:], ALU.add)

                mx = l2.tile([BL, 1], F32)
                nc.vector.tensor_reduce(mx[:], logits[:], axis=AX.X, op=ALU.max)
                sh = l2.tile([BL, C], F32)
                nc.vector.tensor_tensor(sh[:], logits[:],
                                        mx[:].to_broadcast((BL, C)), ALU.subtract)
                ex = l2.tile([BL, C], F32)
                nc.scalar.activation(ex[:], sh[:], ACT.Exp)
                sm = l2.tile([BL, 1], F32)
                nc.vector.tensor_reduce(sm[:], ex[:], axis=AX.X, op=ALU.add)
                lg = l2.tile([BL, 1], F32)
                nc.scalar.activation(lg[:], sm[:], ACT.Ln)
                res = l2.tile([BL, C], F32)
                nc.vector.tensor_tensor(res[:], sh[:],
                                        lg[:].to_broadcast((BL, C)), ALU.subtract)
                nc.sync.dma_start(out_t.ap(), res[:])


def _identity_cos():
    t = np.arange(T)
    f = np.arange(T)
    return np.cos(2.0 * np.pi * np.outer(t, f) / T).astype(np.float32)


def make_inputs(x, edge_index0, edge_index2, W1, b1, W2, b2,
                fc1_w, fc1_b, fc2_w, fc2_b):
    """Build the 8 per-core input maps."""
    A0 = _dense_adj(np.asarray(edge_index0), N0)
    A2 = _dense_adj(np.asarray(edge_index2), N2)
    At1 = np.ascontiguousarray((2.0 * A0).T)   # [j, i] = 2*A0[i, j]
    At2 = np.ascontiguousarray((2.0 * A2).T)

    # resident: rows p*NRES+t = At1 row t*128+p
    at_res = _b16(At1[:JRES].reshape(NRES, 128, N0)
                  .transpose(1, 0, 2).reshape(128 * NRES, N0))
    s = At1[JRES:].reshape(NSTR, 128, N0).transpose(1, 0, 2)  # [p, t, n]
    at_str = np.stack([s[:, :, SL * g:SL * (g + 1)] for g in range(NSL)], 0)
    at_str = _b16(at_str.reshape(NSL * 128 * NSTR, SL))
    a2t = _b16(At2.reshape(NT2, 128, N2).transpose(1, 0, 2).reshape(128 * NT2, N2))

    Ccos = _identity_cos()
    W1e = np.einsum("tf,kfg->ktg", Ccos, np.asarray(W1, np.float32))  # [K, 30, G1]
    w1a = np.zeros((128, K, G1), np.float32)
    for bb in range(4):
        w1a[32 * bb:32 * bb + 30] = W1e.transpose(1, 0, 2)
    w1a = _b16(w1a.reshape(128, K * G1))

    W2f = np.asarray(W2, np.float32)       # [K, G1, G2]
    w2a = np.zeros((128, K, 2, G1), np.float32)
    for bb in range(4):
        for hh in range(2):
            w2a[32 * bb:32 * bb + 32, :, hh, :] = \
                W2f[:, :, 32 * hh:32 * hh + 32].transpose(1, 0, 2)
    w2a = _b16(w2a.reshape(128, K * 2 * G1))

    b1v = np.tile(np.asarray(b1, np.float32), 4).reshape(128, 1)
    b2f = np.asarray(b2, np.float32)
    b2v = np.stack([np.tile(b2f[:32], 4), np.tile(b2f[32:], 4)], 1).astype(np.float32)

    fc1b = np.tile(np.asarray(fc1_b, np.float32)[None, :], (BL, 1))
    fc2b = np.tile(np.asarray(fc2_b, np.float32)[None, :], (BL, 1))
    fc2w = _b16(np.asarray(fc2_w, np.float32))

    xf = np.asarray(x, np.float32)          # [B, N0, T]
    fc1wf = np.asarray(fc1_w, np.float32)   # [N2*G2, D]

    ins = []
    for core in range(NCORES):
        xs = xf[BL * core:BL * (core + 1)]          # [4, N0, 30]
        x_cn = np.zeros((BL, TP, N0), np.float32)
        x_cn[:, :T] = xs.transpose(0, 2, 1)
        x_cn = x_cn.reshape(CH, N0)                 # row = b*32 + t
        x_lt = x_cn.T.reshape(NT0, 128, CH).transpose(1, 0, 2).reshape(128 * NT0, CH)
        # fcw[p=n2l, kt=g2, d] = fc1_w[8192*core + n2l*64 + g2, d]: the
        # natural row-major order of the core's slice, no interleave.
        fc1w_r = fc1wf[FBLK * core:FBLK * (core + 1)]
        ins.append({
            "at_res": at_res, "at_str": at_str, "a2t": a2t,
            "x_cn": _b16(x_cn), "x_lt": _b16(x_lt),
            "w1a": w1a, "w2a": w2a, "b1v": b1v, "b2v": b2v,
            "fc1w": _b16(fc1w_r), "fc1b": fc1b,
            "fc2w": fc2w, "fc2b": fc2b,
        })
    return ins


_CACHED = {}


def kernel(**inputs):
    if "nc" not in _CACHED:
        _CACHED["nc"] = build_program(dbg=False)
    nc = _CACHED["nc"]
    ins = make_inputs(**inputs)
    res = run_bass_kernel_spmd(nc, ins, core_ids=list(range(NCORES)))
    out = np.zeros((B, C), np.float32)
    for core in range(NCORES):
        out[BL * core:BL * (core + 1)] = res.results[core]["out"]
    return out


# revision 19
# speedup vs baseline: 1.8522x; 1.0175x over previous
"""NetTGCN forward pass on 8 Trainium2 NeuronCores (Bass/Tile).

Batch-parallel design, zero collectives until the fc head:
  Each core owns 4 batches. Layer-1 channels = 4 batches x 32 taps = 128 =
  exactly the SBUF partition width, so the full Chebyshev recurrence on the
  4096-node graph runs locally per core: state kept in SBUF in both
  [ch, node] (recurrence/contract) and node-major lhsT form (matmul
  stationary). The dense operator 2A^T (bf16, 33.5 MB) is split: 13 of 32
  contract row-tiles stay SBUF-resident, the other 19 are streamed from HBM
  per 512-column output slice (2.4 MB contiguous DMAs, hidden under the
  matmuls). Per Chebyshev term: 256 matmuls of [128x128]@[128x512] (~99% PE
  eff), 32 PE transposes to rebuild the lhsT form, and an inline W1[k]
  contraction into the fp32 h1 accumulator.
  The FFT is folded into W1 on the host (real(FFT(x)) = x @ Ccos commutes
  with the graph operator).
  Layer 2 (1024-node graph) is identical in structure with the 2 MB
  operator fully resident.
  Head: features are exchanged with one 8-rank AllToAll so each core
  contracts its 8192-row slice of fc1_w for all 32 batches; partial z is
  ReduceScattered (each core gets its own 4 batches), fc2 + log_softmax run
  locally, and the host concatenates per-core outputs.

States are bf16 throughout (validated on host: final rel err 6.4e-3 vs
6.1e-3 for fp32 states); accumulators (h1/h2/psum) are fp32.
"""

import sys

if "/opt/trn_rl_repo" not in sys.path:
    sys.path.insert(0, "/opt/trn_rl_repo")

import numpy as np
import ml_dtypes

import concourse.bacc as bacc
import concourse.mybir as mybir
import concourse.bass_utils as _bu
from concourse.bass_utils import run_bass_kernel_spmd
from concourse.tile import TileContext
from concourse.masks import make_identity

_bu.upload_artifacts = lambda tmpdir: f"file://{tmpdir}"  # no bucket in sandbox

F32 = mybir.dt.float32
BF16 = mybir.dt.bfloat16
AX = mybir.AxisListType
ALU = mybir.AluOpType
ACT = mybir.ActivationFunctionType

B, N0, T, K = 32, 4096, 30, 25
G1, G2, D, C = 32, 64, 512, 10
N2 = N0 // 4
NCORES = 8
BL = B // NCORES       # 4 batches per core
TP = 32                # taps padded 30 -> 32
CH = BL * TP           # 128 layer-1 channels = partition width
NT0 = N0 // 128        # 32 contract tiles (layer 1)
NRES = 14              # operator row-tiles resident in SBUF
NSTR = NT0 - NRES      # 19 streamed row-tiles
JRES = NRES * 128
SL = 512               # output slice width
NSL = N0 // SL         # 8 slices per term
NT2 = N2 // 128        # 8 contract tiles (layer 2)
FBLK = (N2 * G2) // NCORES  # 8192 fc1 contraction rows per core

G8 = [list(range(NCORES))]


def _b16(a):
    return np.ascontiguousarray(a.astype(ml_dtypes.bfloat16))


def _dense_adj(edge_index, n):
    row = edge_index[0].astype(np.int64)
    col = edge_index[1].astype(np.int64)
    deg = np.zeros(n, np.float32)
    np.add.at(deg, row, 1.0)
    dis = np.where(deg > 0, 1.0 / np.sqrt(np.maximum(deg, 1.0)), 0.0).astype(np.float32)
    w = (-dis[row] * dis[col]).astype(np.float32)
    a = np.zeros((n, n), np.float32)
    np.add.at(a, (row, col), w)
    return a


def build_program(dbg=False):
    nc = bacc.Bacc("TRN2", target_bir_lowering=False, debug=False,
                   num_devices=NCORES)

    at_res_in = nc.dram_tensor("at_res", [128 * NRES, N0], BF16, kind="ExternalInput")
    at_str_in = nc.dram_tensor("at_str", [NSL * 128 * NSTR, SL], BF16, kind="ExternalInput")
    x_cn_in = nc.dram_tensor("x_cn", [128, N0], BF16, kind="ExternalInput")
    x_lt_in = nc.dram_tensor("x_lt", [128 * NT0, CH], BF16, kind="ExternalInput")
    w1_in = nc.dram_tensor("w1a", [128, K * G1], BF16, kind="ExternalInput")
    b1_in = nc.dram_tensor("b1v", [128, 1], F32, kind="ExternalInput")
    a2t_in = nc.dram_tensor("a2t", [128 * NT2, N2], BF16, kind="ExternalInput")
    w2_in = nc.dram_tensor("w2a", [128, K * 2 * G1], BF16, kind="ExternalInput")
    b2_in = nc.dram_tensor("b2v", [128, 2], F32, kind="ExternalInput")
    fc1w_in = nc.dram_tensor("fc1w", [128 * (FBLK // 128), D], BF16, kind="ExternalInput")
    fc1b_in = nc.dram_tensor("fc1b", [BL, D], F32, kind="ExternalInput")
    fc2w_in = nc.dram_tensor("fc2w", [D, C], BF16, kind="ExternalInput")
    fc2b_in = nc.dram_tensor("fc2b", [BL, C], F32, kind="ExternalInput")

    out_t = nc.dram_tensor("out", [BL, C], F32, kind="ExternalOutput")
    if dbg:
        h1_dbg = nc.dram_tensor("h1_dbg", [128, N0], F32, kind="ExternalOutput")
        h1p_dbg = nc.dram_tensor("h1p_dbg", [128, N2], F32, kind="ExternalOutput")
        h2_dbg = nc.dram_tensor("h2_dbg", [128, 2 * N2], F32, kind="ExternalOutput")
        z_dbg = nc.dram_tensor("z_dbg", [BL, D], F32, kind="ExternalOutput")

    cch_in = nc.dram_tensor("cch_in", [NCORES * 128 * 64, BL], BF16)
    cch_out = nc.dram_tensor("cch_out", [NCORES * 128 * 64, BL], BF16)
    ccz_in = nc.dram_tensor("ccz_in", [B, D], F32)
    ccz_out = nc.dram_tensor("ccz_out", [BL, D], F32)

    with TileContext(nc) as tc:
        with tc.tile_pool(name="const", bufs=1) as cpool:
            identb = cpool.tile([128, 128], BF16)
            make_identity(nc, identb[:])
            identf = cpool.tile([128, 128], F32)
            make_identity(nc, identf[:])
            h1_sb = cpool.tile([128, N0], F32)
            h1p = cpool.tile([128, N2], F32)

            # ======================= LAYER 1 =======================
            with tc.tile_pool(name="l1a", bufs=1) as l1a, \
                 tc.tile_pool(name="l1s", bufs=3) as l1s, \
                 tc.tile_pool(name="l1st", bufs=1) as l1st, \
                 tc.tile_pool(name="ps_y", bufs=2, space="PSUM") as ps_y, \
                 tc.tile_pool(name="ps_tr", bufs=4, space="PSUM") as ps_tr, \
                 tc.tile_pool(name="ps_h", bufs=2, space="PSUM") as ps_h:

                # small inputs first: the DMA rings are FIFO, so the x /
                # weight loads must not queue behind 13 MB of operator tiles
                w1a = l1a.tile([128, K, G1], BF16)
                nc.sync.dma_start(w1a[:], w1_in.ap().rearrange("p (k g) -> p k g", k=K))
                b1v = l1a.tile([128, 1], F32)
                nc.sync.dma_start(b1v[:], b1_in.ap())
                cn = [l1st.tile([128, N0], BF16, name=f"cn{i}", tag=f"cn{i}")
                      for i in range(2)]
                lt = [l1st.tile([128, NT0, CH], BF16, name=f"lt{i}", tag=f"lt{i}")
                      for i in range(2)]
                nc.sync.dma_start(cn[0][:], x_cn_in.ap())
                nc.sync.dma_start(lt[0][:],
                                  x_lt_in.ap().rearrange("(p t) c -> p t c", t=NT0))

                at_res = l1a.tile([128, NRES, N0], BF16)
                at_res_v = at_res_in.ap().rearrange("(p t) n -> p t n", t=NRES)
                for t in range(NRES):
                    nc.sync.dma_start(at_res[:, t, :], at_res_v[:, t, :])

                at_str_v = at_str_in.ap().rearrange(
                    "(g p t) n -> g p t n", g=NSL, p=128)

                def contract1(src_cn, kk, g, first):
                    sl = slice(SL * g, SL * (g + 1))
                    hp = ps_h.tile([128, SL], F32, tag="hp", name=f"hp{kk}_{g}")
                    for bb in range(4):
                        nc.tensor.matmul(
                            hp[32 * bb:32 * (bb + 1), :],
                            w1a[32 * bb:32 * (bb + 1), kk, :],
                            src_cn[32 * bb:32 * (bb + 1), sl],
                            start=True, stop=True,
                            tile_position=(32 * bb, 32 * bb))
                    if first:
                        nc.vector.tensor_copy(h1_sb[:, sl], hp[:])
                    else:
                        nc.vector.tensor_tensor(h1_sb[:, sl], h1_sb[:, sl],
                                                hp[:], ALU.add)

                for g in range(NSL):
                    contract1(cn[0], 0, g, True)

                def epi1(k, g):
                    """Transposes + h1 contract for slice (k, g); emitted
                    after the NEXT slice's matmuls so PE never stalls on
                    the DVE recurrence."""
                    new_cn = cn[k % 2]
                    new_lt = lt[k % 2]
                    for bq in range(4):
                        trp = ps_tr.tile([128, 128], BF16, tag="trp",
                                         name=f"trp{k}_{g}_{bq}")
                        nc.tensor.transpose(
                            trp[:],
                            new_cn[:, SL * g + 128 * bq:SL * g + 128 * (bq + 1)],
                            identb[:])
                        nc.vector.tensor_copy(new_lt[:, 4 * g + bq, :], trp[:])
                    contract1(new_cn, k, g, False)

                for k in range(1, K):
                    cur_lt = lt[(k - 1) % 2]
                    # in-place ring: tx_k overwrites tx_{k-2} slice by slice
                    new_cn = cn[k % 2]
                    prev_cn = new_cn if k >= 2 else None
                    for g in range(NSL):
                        sl = slice(SL * g, SL * (g + 1))
                        sa = l1s.tile([128, 9, SL], BF16, tag="sa",
                                      name=f"sa{k}_{g}")
                        sb = l1s.tile([128, 9, SL], BF16, tag="sa",
                                      name=f"sb{k}_{g}")
                        nc.sync.dma_start(sa[:], at_str_v[g][:, 0:9, :])
                        nc.sync.dma_start(sb[:], at_str_v[g][:, 9:18, :])
                        yp = ps_y.tile([128, SL], F32, tag="yp", name=f"yp{k}_{g}")
                        for jt in range(NRES):
                            nc.tensor.matmul(yp[:], cur_lt[:, jt, :],
                                             at_res[:, jt, sl],
                                             start=(jt == 0), stop=False)
                        for t in range(9):
                            nc.tensor.matmul(yp[:], cur_lt[:, NRES + t, :],
                                             sa[:, t, :],
                                             start=False, stop=False)
                        for t in range(9):
                            nc.tensor.matmul(yp[:], cur_lt[:, NRES + 9 + t, :],
                                             sb[:, t, :],
                                             start=False, stop=(t == 8))
                        if k == 1:
                            nc.vector.tensor_scalar_mul(new_cn[:, sl], yp[:], 0.5)
                        else:
                            nc.vector.tensor_tensor(new_cn[:, sl], yp[:],
                                                    prev_cn[:, sl], ALU.subtract)
                        if g > 0:
                            epi1(k, g - 1)
                    epi1(k, NSL - 1)

                # bias + relu + maxpool4 along nodes
                for q in range(4):
                    nc.scalar.activation(h1_sb[:, 1024 * q:1024 * (q + 1)],
                                         h1_sb[:, 1024 * q:1024 * (q + 1)],
                                         ACT.Relu, bias=b1v[:])
                if dbg:
                    nc.sync.dma_start(h1_dbg.ap(), h1_sb[:])
                h4 = h1_sb[:].rearrange("p (n f) -> p n f", f=4)
                nc.vector.tensor_tensor(h1p[:], h4[:, :, 0], h4[:, :, 1], ALU.max)
                nc.vector.tensor_tensor(h1p[:], h1p[:], h4[:, :, 2], ALU.max)
                nc.vector.tensor_tensor(h1p[:], h1p[:], h4[:, :, 3], ALU.max)
                if dbg:
                    nc.sync.dma_start(h1p_dbg.ap(), h1p[:])

            # ======================= LAYER 2 =======================
            with tc.tile_pool(name="l2", bufs=1) as l2, \
                 tc.tile_pool(name="l2st", bufs=1) as l2st, \
                 tc.tile_pool(name="ps2t", bufs=2, space="PSUM") as ps2t:

                a2t = l2.tile([128, NT2, N2], BF16)
                a2t_v = a2t_in.ap().rearrange("(p t) n -> p t n", t=NT2)
                for t in range(NT2):
                    nc.sync.dma_start(a2t[:, t, :], a2t_v[:, t, :])
                w2a = l2.tile([128, K, 2, G1], BF16)
                nc.sync.dma_start(
                    w2a[:], w2_in.ap().rearrange("p (k h g) -> p k h g", k=K, h=2))
                b2v = l2.tile([128, 2], F32)
                nc.sync.dma_start(b2v[:], b2_in.ap())
                fcw = l2.tile([128, FBLK // 128, D], BF16)
                fcw_v = fc1w_in.ap().rearrange("(p t) d -> p t d", t=FBLK // 128)
                for q in range(8):
                    nc.sync.dma_start(fcw[:, 8 * q:8 * (q + 1), :],
                                      fcw_v[:, 8 * q:8 * (q + 1), :])

                cn2 = [l2st.tile([128, N2], BF16, name=f"cn2_{i}", tag=f"cn2_{i}")
                       for i in range(3)]
                lt2 = [l2st.tile([128, NT2, CH], BF16, name=f"lt2_{i}",
                                 tag=f"lt2_{i}") for i in range(2)]
                h2r = l2.tile([128, 2, N2], F32)
                ft = l2.tile([128, NT2, G2, BL], BF16)

                with tc.tile_pool(name="ps2y", bufs=2, space="PSUM") as ps2y, \
                     tc.tile_pool(name="ps2h", bufs=1, space="PSUM") as ps2h:
                    # h2 accumulates in PSUM across all K terms: 4 banks,
                    # start at k=0, stop at k=K-1, relu reads PSUM directly.
                    h2ps = [[ps2h.tile([128, SL], F32, tag=f"h2ps_{hh}_{g}",
                                       name=f"h2ps_{hh}_{g}")
                             for g in range(2)] for hh in range(2)]

                    nc.vector.tensor_copy(cn2[0][:], h1p[:])
                    for nt in range(NT2):
                        trp = ps2t.tile([128, 128], BF16, tag="tr2",
                                        name=f"tr2i_{nt}")
                        nc.tensor.transpose(
                            trp[:], cn2[0][:, 128 * nt:128 * (nt + 1)], identb[:])
                        nc.vector.tensor_copy(lt2[0][:, nt, :], trp[:])

                    def contract2(src_cn, kk, g):
                        sl = slice(SL * g, SL * (g + 1))
                        for hh in range(2):
                            for bb in range(4):
                                nc.tensor.matmul(
                                    h2ps[hh][g][32 * bb:32 * (bb + 1), :],
                                    w2a[32 * bb:32 * (bb + 1), kk, hh, :],
                                    src_cn[32 * bb:32 * (bb + 1), sl],
                                    start=(kk == 0), stop=(kk == K - 1),
                                    tile_position=(32 * bb, 32 * bb))

                    def epi2(k, g):
                        new_cn2 = cn2[k % 3]
                        new_lt2 = lt2[k % 2]
                        for bq in range(4):
                            trp = ps2t.tile([128, 128], BF16, tag="tr2",
                                            name=f"tr2_{k}_{g}_{bq}")
                            nc.tensor.transpose(
                                trp[:],
                                new_cn2[:, SL * g + 128 * bq:SL * g + 128 * (bq + 1)],
                                identb[:])
                            nc.vector.tensor_copy(new_lt2[:, 4 * g + bq, :], trp[:])
                        contract2(new_cn2, k, g)

                    contract2(cn2[0], 0, 0)
                    contract2(cn2[0], 0, 1)
                    for k in range(1, K):
                        cur_lt2 = lt2[(k - 1) % 2]
                        new_cn2 = cn2[k % 3]
                        prev_cn2 = cn2[(k - 2) % 3] if k >= 2 else None
                        for g in range(2):
                            sl = slice(SL * g, SL * (g + 1))
                            y2 = ps2y.tile([128, SL], F32, tag="y2",
                                           name=f"y2_{k}_{g}")
                            for jt in range(NT2):
                                nc.tensor.matmul(y2[:], cur_lt2[:, jt, :],
                                                 a2t[:, jt, sl],
                                                 start=(jt == 0),
                                                 stop=(jt == NT2 - 1))
                            if k == 1:
                                nc.vector.tensor_scalar_mul(new_cn2[:, sl],
                                                            y2[:], 0.5)
                            else:
                                nc.vector.tensor_tensor(new_cn2[:, sl], y2[:],
                                                        prev_cn2[:, sl],
                                                        ALU.subtract)
                            if g > 0:
                                epi2(k, g - 1)
                        epi2(k, 1)

                    # ================== HEAD (part 1) ==================
                    for hh in range(2):
                        for g in range(2):
                            sl = slice(SL * g, SL * (g + 1))
                            nc.scalar.activation(h2r[:, hh, sl], h2ps[hh][g][:],
                                                 ACT.Relu, bias=b2v[:, hh:hh + 1])
                    if dbg:
                        nc.sync.dma_start(
                            h2_dbg.ap().rearrange("p (h n) -> p h n", h=2),
                            h2r[:])

                    # features to f-major: ft[n2l, nt, g2, b]
                    for hh in range(2):
                        for nt in range(NT2):
                            trp = ps2t.tile([128, 128], F32, tag="tr2",
                                            name=f"trh_{hh}_{nt}")
                            nc.tensor.transpose(
                                trp[:], h2r[:, hh, 128 * nt:128 * (nt + 1)],
                                identf[:])
                            nc.vector.tensor_copy(
                                out=ft[:, nt, 32 * hh:32 * (hh + 1), :],
                                in_=trp[:].rearrange("p (b g) -> p g b", b=BL))
                    nc.sync.dma_start(
                        cch_in.ap().rearrange("(j nl g) b -> nl j (g b)",
                                              j=NCORES, nl=128),
                        ft[:])
                    nc.gpsimd.collective_compute(
                        "AllToAll", ALU.bypass, replica_groups=G8,
                        ins=[cch_in.ap()], outs=[cch_out.ap()])

                # ================== HEAD (part 2) ==================
                # fc1 partial: z[32, D] for my f-slice. flt is
                # [p=n2l, r, q=g2, b] so each per-rank DMA from cch_out is
                # contiguous on both sides; the matmul stationary reads the
                # strided [p, (r, b)] AP per g2.
                with tc.tile_pool(name="psz", bufs=1, space="PSUM") as psz, \
                     tc.tile_pool(name="psl", bufs=1, space="PSUM") as psl:
                    head_tail(nc, tc, l2, ps2t, psz, psl, identf,
                              cch_out, ccz_in, ccz_out, fcw,
                              fc1b_in, fc2w_in, fc2b_in, out_t,
                              z_dbg if dbg else None)

    nc.compile()
    return nc


def head_tail(nc, tc, l2, ps2t, psz, psl, identf, cch_out, ccz_in, ccz_out,
              fcw, fc1b_in, fc2w_in, fc2b_in, out_t, z_dbg):
    """fc1 partial + ReduceScatter + fc2 + log_softmax."""
    if True:
        if True:
            if True:
                flt = l2.tile([128, NCORES, G2, BL], BF16)
                cch_v = cch_out.ap().rearrange(
                    "(r p q) b -> r p q b", r=NCORES, p=128)
                for r in range(NCORES):
                    nc.sync.dma_start(flt[:, r, :, :], cch_v[r])
                flt2 = l2.tile([128, G2, B], BF16)
                nc.vector.tensor_copy(
                    out=flt2[:].rearrange("p q (r b) -> p q r b", r=NCORES),
                    in_=flt[:].rearrange("p r q b -> p q r b"))
                zps = psz.tile([32, D], F32)
                for kt in range(G2):
                    nc.tensor.matmul(zps[:], flt2[:, kt, :], fcw[:, kt, :],
                                     start=(kt == 0), stop=(kt == G2 - 1))
                zblk = l2.tile([32, D], F32)
                nc.vector.tensor_copy(zblk[:], zps[:])
                nc.sync.dma_start(ccz_in.ap(), zblk[:])
                nc.gpsimd.collective_compute(
                    "ReduceScatter", ALU.add, replica_groups=G8,
                    ins=[ccz_in.ap()], outs=[ccz_out.ap()])
                zfull = l2.tile([BL, D], F32)
                nc.sync.dma_start(zfull[:], ccz_out.ap())
                zb = l2.tile([BL, D], F32)
                nc.sync.dma_start(zb[:], fc1b_in.ap())
                nc.vector.tensor_tensor(zfull[:], zfull[:], zb[:], ALU.add)
                zr = l2.tile([BL, D], F32)
                nc.scalar.activation(zr[:], zfull[:], ACT.Relu)
                if z_dbg is not None:
                    nc.sync.dma_start(z_dbg.ap(), zr[:])

                # fc2 + log_softmax on my 4 batches
                f2w = l2.tile([128, 4, C], BF16)
                nc.sync.dma_start(f2w[:],
                                  fc2w_in.ap().rearrange("(t p) c -> p t c", p=128))
                lps = psl.tile([BL, C], F32)
                for t4 in range(4):
                    ztp = ps2t.tile([128, BL], F32, tag="tr2", name=f"zt_{t4}")
                    nc.tensor.transpose(ztp[:], zr[:, 128 * t4:128 * (t4 + 1)],
                                        identf[:BL, :BL])
                    zts = l2.tile([128, BL], BF16, tag="zts", name=f"zts_{t4}")
                    nc.any.tensor_copy(out=zts[:], in_=ztp[:])
                    nc.tensor.matmul(lps[:], zts[:], f2w[:, t4, :],
                                     start=(t4 == 0), stop=(t4 == 3))
                logits = l2.tile([BL, C], F32)
                f2b = l2.tile([BL, C], F32)
                nc.sync.dma_start(f2b[:], fc2b_in.ap())
                nc.vector.tensor_tensor(logits[:], lps[:], f2b[:], ALU.add)

                mx = l2.tile([BL, 1], F32)
                nc.vector.tensor_reduce(mx[:], logits[:], axis=AX.X, op=ALU.max)
                sh = l2.tile([BL, C], F32)
                nc.vector.tensor_tensor(sh[:], logits[:],
                                        mx[:].to_broadcast((BL, C)), ALU.subtract)
                ex = l2.tile([BL, C], F32)
                nc.scalar.activation(ex[:], sh[:], ACT.Exp)
                sm = l2.tile([BL, 1], F32)
                nc.vector.tensor_reduce(sm[:], ex[:], axis=AX.X, op=ALU.add)
                lg = l2.tile([BL, 1], F32)
                nc.scalar.activation(lg[:], sm[:], ACT.Ln)
                res = l2.tile([BL, C], F32)
                nc.vector.tensor_tensor(res[:], sh[:],
                                        lg[:].to_broadcast((BL, C)), ALU.subtract)
                nc.sync.dma_start(out_t.ap(), res[:])


def _identity_cos():
    t = np.arange(T)
    f = np.arange(T)
    return np.cos(2.0 * np.pi * np.outer(t, f) / T).astype(np.float32)


def make_inputs(x, edge_index0, edge_index2, W1, b1, W2, b2,
                fc1_w, fc1_b, fc2_w, fc2_b):
    """Build the 8 per-core input maps."""
    A0 = _dense_adj(np.asarray(edge_index0), N0)
    A2 = _dense_adj(np.asarray(edge_index2), N2)
    At1 = np.ascontiguousarray((2.0 * A0).T)   # [j, i] = 2*A0[i, j]
    At2 = np.ascontiguousarray((2.0 * A2).T)

    # resident: rows p*NRES+t = At1 row t*128+p
    at_res = _b16(At1[:JRES].reshape(NRES, 128, N0)
                  .transpose(1, 0, 2).reshape(128 * NRES, N0))
    s = At1[JRES:].reshape(NSTR, 128, N0).transpose(1, 0, 2)  # [p, t, n]
    at_str = np.stack([s[:, :, SL * g:SL * (g + 1)] for g in range(NSL)], 0)
    at_str = _b16(at_str.reshape(NSL * 128 * NSTR, SL))
    a2t = _b16(At2.reshape(NT2, 128, N2).transpose(1, 0, 2).reshape(128 * NT2, N2))

    Ccos = _identity_cos()
    W1e = np.einsum("tf,kfg->ktg", Ccos, np.asarray(W1, np.float32))  # [K, 30, G1]
    w1a = np.zeros((128, K, G1), np.float32)
    for bb in range(4):
        w1a[32 * bb:32 * bb + 30] = W1e.transpose(1, 0, 2)
    w1a = _b16(w1a.reshape(128, K * G1))

    W2f = np.asarray(W2, np.float32)       # [K, G1, G2]
    w2a = np.zeros((128, K, 2, G1), np.float32)
    for bb in range(4):
        for hh in range(2):
            w2a[32 * bb:32 * bb + 32, :, hh, :] = \
                W2f[:, :, 32 * hh:32 * hh + 32].transpose(1, 0, 2)
    w2a = _b16(w2a.reshape(128, K * 2 * G1))

    b1v = np.tile(np.asarray(b1, np.float32), 4).reshape(128, 1)
    b2f = np.asarray(b2, np.float32)
    b2v = np.stack([np.tile(b2f[:32], 4), np.tile(b2f[32:], 4)], 1).astype(np.float32)

    fc1b = np.tile(np.asarray(fc1_b, np.float32)[None, :], (BL, 1))
    fc2b = np.tile(np.asarray(fc2_b, np.float32)[None, :], (BL, 1))
    fc2w = _b16(np.asarray(fc2_w, np.float32))

    xf = np.asarray(x, np.float32)          # [B, N0, T]
    fc1wf = np.asarray(fc1_w, np.float32)   # [N2*G2, D]

    ins = []
    for core in range(NCORES):
        xs = xf[BL * core:BL * (core + 1)]          # [4, N0, 30]
        x_cn = np.zeros((BL, TP, N0), np.float32)
        x_cn[:, :T] = xs.transpose(0, 2, 1)
        x_cn = x_cn.reshape(CH, N0)                 # row = b*32 + t
        x_lt = x_cn.T.reshape(NT0, 128, CH).transpose(1, 0, 2).reshape(128 * NT0, CH)
        # fcw[p=n2l, kt=g2, d] = fc1_w[8192*core + n2l*64 + g2, d]: the
        # natural row-major order of the core's slice, no interleave.
        fc1w_r = fc1wf[FBLK * core:FBLK * (core + 1)]
        ins.append({
            "at_res": at_res, "at_str": at_str, "a2t": a2t,
            "x_cn": _b16(x_cn), "x_lt": _b16(x_lt),
            "w1a": w1a, "w2a": w2a, "b1v": b1v, "b2v": b2v,
            "fc1w": _b16(fc1w_r), "fc1b": fc1b,
            "fc2w": fc2w, "fc2b": fc2b,
        })
    return ins


_CACHED = {}


def kernel(**inputs):
    if "nc" not in _CACHED:
        _CACHED["nc"] = build_program(dbg=False)
    nc = _CACHED["nc"]
    ins = make_inputs(**inputs)
    res = run_bass_kernel_spmd(nc, ins, core_ids=list(range(NCORES)))
    out = np.zeros((B, C), np.float32)
    for core in range(NCORES):
        out[BL * core:BL * (core + 1)] = res.results[core]["out"]
    return out
